# revision 80
# baseline (speedup 1.0000x reference)
"""Trainium2 Bass kernel for nn_Block_55336358643145 (dense transformer block).

Row-shards the 4096 (batch*seq) rows across 8 NeuronCores, 512 rows per core as
two 256-row blocks of the SAME batch (cores 0-3 carry batch 0 positions
(p, 7-p); cores 4-7 mirror for batch 1), so both the k and v AllGathers run as
two independent 4-core groups and both q-blocks of a core read the same
gathered kv stream. Per core: LN1 -> transpose -> Q/K/V (q,k head-transposed,
v row-layout with a per-head ones column so the softmax denominator falls out
of the attn@v matmul), chunked grouped AllGathers of k/v (k_a, v_a, k_b, v_b)
pipelined against production and consumption, diagonal (own-kv) attention from
SBUF-resident kT/vaug overlapped with the collectives, then a uniform 14-slot
kv sweep where each 128-row kv tile hits both q-blocks in one 512-wide matmul
(per-core bias columns kill the halves that are causally out of range:
exp == 0 exactly), attn@v, out-proj + residual, LN2, 4x MLP with exact-erf
Gelu and a qt-major down-proj against SBUF-resident W2 so the tail drains
fast. Host reassembles.
"""

import contextlib

import numpy as np

import concourse.bass as bass
import concourse.tile as tile
from concourse import bacc, mybir
from concourse.bass_utils import run_bass_kernel_spmd

F32 = mybir.dt.float32
F32R = mybir.dt.float32r
AF = mybir.ActivationFunctionType
ALU = mybir.AluOpType

B, S, D, H, HD, FF = 2, 2048, 1024, 16, 64, 4096
NCORE = 8
R = 512            # rows per core
QB = 256           # q rows per block
NBLK = 2           # blocks per core
KTW = 128          # kpos tile width
NSLOT = 14         # pass-A kv tile slots (uniform across cores)
LN_EPS = 1e-5
JD = 25            # joined dim for the column-zero mask
NEG = -1.0e30
DC = D // 128      # 8 d-chunks
GC = FF // 128     # 32 mlp hidden chunks
VW = H * (HD + 1)  # 1040: v with per-head ones column


BF16 = mybir.dt.bfloat16


def _gtile_src(t):
    """128-wide kv tile t of this batch -> (group rank, col/row offset)."""
    p = t // 2
    if p <= 3:
        return p, 128 * (t % 2)          # position p -> rank p, block0 half
    return 7 - p, 256 + 128 * (t % 2)    # position p -> rank 7-p, block1 half


def build_program(apply_bv, apply_ln1_gb, apply_ln2_gb):
    nc = bacc.Bacc("TRN2", target_bir_lowering=False, debug=False,
                   num_devices=NCORE)

    def inp(name, shape):
        return nc.dram_tensor(name, list(shape), F32, kind="ExternalInput").ap()

    def binp(name, shape):
        return nc.dram_tensor(name, list(shape), BF16,
                              kind="ExternalInput").ap()

    io = dict(
        hs=inp("hs", (R, D)),
        wq=binp("wq", (D, D)), wk=binp("wk", (D, D)),
        wv=binp("wv", (D, D)), wp=binp("wp", (D, D)),
        w1=binp("w1", (GC, 128, DC, 128)), w2=binp("w2", (FF, D)),
        bq8=inp("bq8", (128, DC)), bkl=inp("bkl", (128, DC)),
        bvh=inp("bvh", (HD, H)), b1l=inp("b1l", (128, GC)),
        bpr=binp("bpr", (1, D)), b2r=binp("b2r", (1, D)),
        ln1gb=inp("ln1gb", (2, D)), ln2gb=inp("ln2gb", (2, D)),
        biasA=inp("biasA", (128, NSLOT, 2)),
        biasB=inp("biasB", (128, NBLK, 2)),
        maskAB=inp("maskAB", (2, 128, 2 * KTW)),
        ident=binp("ident", (128, 128)),
        onesr=binp("onesr", (1, 128)),
        out=nc.dram_tensor("out", [R, D], F32, kind="ExternalOutput").ap(),
    )

    with tile.TileContext(nc) as tc:
        _build(tc, io, apply_bv, apply_ln1_gb, apply_ln2_gb)
    nc.compile()
    return nc


def _build(tc, io, apply_bv, apply_ln1_gb, apply_ln2_gb):
    nc = tc.nc
    hs, out = io["hs"], io["out"]

    with contextlib.ExitStack() as ctx:
        persist = ctx.enter_context(tc.tile_pool(name="persist", bufs=1, side="left"))
        dram = ctx.enter_context(tc.tile_pool(name="dram", bufs=1,
                                              space="DRAM"))

        # ---- P0-critical constants only; the rest load later ----------------
        eps_sb = persist.tile([128, 1], F32)
        nc.vector.memset(eps_sb[:], LN_EPS)
        ident_sb = persist.tile([128, 128], BF16)
        nc.sync.dma_start(ident_sb[:], io["ident"][:])

        # projection weight tiles; wk's load is issued here (first consumer is
        # P1), wv/wq loads are issued after P0 so the hs tiles win the queue
        es_w = ctx.enter_context(contextlib.ExitStack())      # wk/wv/wq: ..P3
        wqkv_pool = es_w.enter_context(
            tc.tile_pool(name="wqkv_p", bufs=1, side="left"))
        wqkv = {}
        for nm in ("wk", "wv", "wq"):
            wqkv[nm] = wqkv_pool.tile([128, DC, D], BF16, name=f"w_{nm}")

        def ln_gb_tiles(gb_inp, nm):
            g_sb = persist.tile([128, D], F32, name=f"g_{nm}")
            b_sb = persist.tile([128, D], F32, name=f"b_{nm}")
            g_row = persist.tile([1, D], F32, name=f"gr_{nm}")
            b_row = persist.tile([1, D], F32, name=f"br_{nm}")
            nc.sync.dma_start(g_row[:], gb_inp[0:1, :])
            nc.sync.dma_start(b_row[:], gb_inp[1:2, :])
            nc.gpsimd.partition_broadcast(g_sb[:], g_row[:])
            nc.gpsimd.partition_broadcast(b_sb[:], b_row[:])
            return g_sb, b_sb

        ln1_g = ln1_b = ln2_g = ln2_b = None
        if apply_ln1_gb:
            ln1_g, ln1_b = ln_gb_tiles(io["ln1gb"], "ln1")

        def layernorm(dst, src, pool, g_sb, b_sb):
            stats = pool.tile([128, 2, 6], F32, tag="ln_stats")
            sg = src.rearrange("p (g d) -> p g d", g=2)
            for g in range(2):
                nc.vector.bn_stats(out=stats[:, g, :], in_=sg[:, g, :])
            mv = pool.tile([128, 2], F32, tag="ln_mv")
            nc.vector.bn_aggr(out=mv[:], in_=stats[:])
            rstd = pool.tile([128, 1], F32, tag="ln_rstd")
            nc.scalar.activation(out=rstd[:], in_=mv[:, 1:2], func=AF.Sqrt,
                                 bias=eps_sb[:], scale=1.0)
            nc.vector.reciprocal(out=rstd[:], in_=rstd[:])
            nc.vector.tensor_scalar(out=dst, in0=src, scalar1=mv[:, 0:1],
                                    scalar2=rstd[:], op0=ALU.subtract,
                                    op1=ALU.mult)
            if g_sb is not None:
                nc.vector.tensor_mul(dst, dst, g_sb[:])
                nc.vector.tensor_add(dst, dst, b_sb[:])

        def transpose_into(dstT, src_tile, rt, tp_pool):
            for c in range(DC):
                tp = tp_pool.tile([128, 128], BF16, tag="tp")
                nc.tensor.transpose(tp[:], src_tile[:, 128 * c:128 * (c + 1)],
                                    ident_sb[:])
                nc.scalar.copy(dstT[:, c, 128 * rt:128 * (rt + 1)], tp[:])

        es_x = ctx.enter_context(contextlib.ExitStack())      # xT: P0..P3
        es_q = ctx.enter_context(contextlib.ExitStack())      # qT: P3..P4
        xT_pool = es_x.enter_context(
            tc.tile_pool(name="xT_p", bufs=1, side="left"))
        qT_pool = es_q.enter_context(
            tc.tile_pool(name="qT_p", bufs=1, side="right"))
        xT = xT_pool.tile([128, DC, R], BF16)
        qT = qT_pool.tile([128, DC, R], BF16)

        # ================= P0: load + LN1 + transpose ========================
        # first two hs tiles win the DMA queue; wk (needed only at P1)
        # transfers behind them
        with tc.tile_pool(name="p0", bufs=2, side="left") as p0, \
             tc.tile_pool(name="p0ps", bufs=4, space="PSUM") as p0ps:
            hst_head = []
            for rt in range(2):
                hst = p0.tile([128, D], F32, tag="hst")
                nc.sync.dma_start(hst[:], hs[128 * rt:128 * (rt + 1), :])
                hst_head.append(hst)
            nc.sync.dma_start(
                wqkv["wk"][:],
                io["wk"][:].rearrange("(c p) d -> p c d", p=128))
            for rt in range(4):
                if rt < 2:
                    hst = hst_head[rt]
                else:
                    hst = p0.tile([128, D], F32, tag="hst")
                    nc.sync.dma_start(hst[:],
                                      hs[128 * rt:128 * (rt + 1), :])
                xln = p0.tile([128, D], BF16, tag="xln")
                layernorm(xln[:], hst[:], p0, ln1_g, ln1_b)
                transpose_into(xT, xln, rt, p0ps)
        nc.sync.dma_start(
            wqkv["wv"][:], io["wv"][:].rearrange("(c p) d -> p c d", p=128))

        # ---- remaining small constants (off the P0 critical path) ----------
        bq8_sb = persist.tile([128, DC], F32)
        nc.sync.dma_start(bq8_sb[:], io["bq8"][:])
        bkl_sb = persist.tile([128, DC], F32)
        nc.sync.dma_start(bkl_sb[:], io["bkl"][:])
        b1l_sb = persist.tile([128, GC], F32)
        nc.sync.dma_start(b1l_sb[:], io["b1l"][:])
        ones_r = persist.tile([1, 128], BF16)
        nc.sync.dma_start(ones_r[:], io["onesr"][:])
        bpr_sb = persist.tile([1, D], BF16)
        nc.sync.dma_start(bpr_sb[:], io["bpr"][:])
        b2r_sb = persist.tile([1, D], BF16)
        nc.sync.dma_start(b2r_sb[:], io["b2r"][:])
        biasA_sb = persist.tile([128, NSLOT, 2], F32)
        nc.sync.dma_start(biasA_sb[:], io["biasA"][:])
        biasB_sb = persist.tile([128, NBLK, 2], F32)
        nc.sync.dma_start(biasB_sb[:], io["biasB"][:])
        maskA_sb = persist.tile([128, 2, 2 * KTW], F32)
        for j in range(2):
            nc.sync.dma_start(maskA_sb[:, j, :], io["maskAB"][0])
        maskA_sb = maskA_sb[:].rearrange("p a b -> p (a b)")
        maskB_sb = persist.tile([128, 2, 2 * KTW], F32)
        for j in range(2):
            nc.sync.dma_start(maskB_sb[:, j, :], io["maskAB"][1])
        maskB_sb = maskB_sb[:].rearrange("p a b -> p (a b)")
        if apply_bv:
            bvh_sb = persist.tile([HD, H], F32)
            nc.sync.dma_start(bvh_sb[:], io["bvh"][:])
        ln2_g = ln2_b = None
        if apply_ln2_gb:
            ln2_g, ln2_b = ln_gb_tiles(io["ln2gb"], "ln2")

        def proj_headT(dstT, wt, bias_sb, scale, nm):
            """dstT[:, oc, :] = ((x @ w) * scale + bias)^T rows 128oc..+128."""
            with tc.tile_pool(name=f"ps_{nm}", bufs=2, space="PSUM") as pps:
                for oc in range(DC):
                    ps = pps.tile([128, R], F32, tag="ps", name=f"ps_{nm}_{oc}")
                    for c in range(DC):
                        nc.tensor.matmul(
                            ps[:], wt[:, c, 128 * oc:128 * (oc + 1)],
                            xT[:, c, :], start=(c == 0), stop=(c == DC - 1))
                    nc.scalar.activation(dstT[:, oc, :], ps[:],
                                         func=AF.Identity,
                                         bias=bias_sb[:, oc:oc + 1],
                                         scale=scale)

        # DRAM bounce + gathered buffers for the grouped (4-way) k/v AllGathers
        k_loc = dram.tile([D, R], BF16)
        v_loc = dram.tile([R, VW], BF16)
        k_g = dram.tile([4, D, R], BF16)
        v_g = dram.tile([4, R, VW], BF16)
        GROUPS = [[0, 1, 2, 3], [4, 5, 6, 7]]

        es_kv = ctx.enter_context(contextlib.ExitStack())     # kT/vaug: ..P4a
        kT_pool = es_kv.enter_context(
            tc.tile_pool(name="kT_p", bufs=1, side="right"))
        vaug_pool = es_kv.enter_context(
            tc.tile_pool(name="vaug_p", bufs=1, side="right"))
        kT = kT_pool.tile([128, DC, R], BF16)
        vaug = vaug_pool.tile([128, 4, VW], BF16)

        # ================= P1: kT; store + AllGather(k) ======================
        proj_headT(kT, wqkv["wk"], bkl_sb, 1.0, "wk")
        nc.sync.dma_start(k_loc[:].rearrange("(c p) q -> p c q", p=128),
                          kT[:])
        nc.gpsimd.collective_compute(
            "AllGather", ALU.bypass, replica_groups=GROUPS,
            ins=[k_loc.opt()], outs=[k_g.opt()])
        # wq load deliberately after the k store + AllGather trigger: it is
        # not needed until P3 and would delay the collective on the DMA queue
        nc.sync.dma_start(
            wqkv["wq"][:], io["wq"][:].rearrange("(c p) d -> p c d", p=128))

        # ================= P2: v rows + ones cols; AllGather(v) ==============
        with tc.tile_pool(name="ps_wv", bufs=2, space="PSUM") as pps:
            # per-head softmax-denominator ones column
            nc.gpsimd.memset(
                vaug[:].rearrange("p f (h e) -> p f h e", e=HD + 1)
                [:, :, :, HD:HD + 1], 1.0)
            wt = wqkv["wv"]
            for pt in range(4):
                for cg in range(2):
                    ps = pps.tile([128, 512], F32, tag="ps",
                                  name=f"ps_wv_{pt}_{cg}")
                    for c in range(DC):
                        nc.tensor.matmul(
                            ps[:], xT[:, c, 128 * pt:128 * (pt + 1)],
                            wt[:, c, 512 * cg:512 * (cg + 1)],
                            start=(c == 0), stop=(c == DC - 1))
                    for hh in range(8):
                        h = 8 * cg + hh
                        nc.scalar.copy(
                            vaug[:, pt, (HD + 1) * h:(HD + 1) * h + HD],
                            ps[:, HD * hh:HD * (hh + 1)])
            nc.sync.dma_start(v_loc[:].rearrange("(f p) w -> p f w", p=128),
                              vaug[:])
            nc.gpsimd.collective_compute(
                "AllGather", ALU.bypass, replica_groups=GROUPS,
                ins=[v_loc.opt()], outs=[v_g.opt()])

        # ================= P3: qT (scaled by 1/8) ============================
        proj_headT(qT, wqkv["wq"], bq8_sb, 0.125, "wq")
        es_x.close()  # xT no longer needed
        es_w.close()  # wk/wv/wq no longer needed

        # ================= P4: attention =====================================
        es_attn = ctx.enter_context(contextlib.ExitStack())   # attn_oT: P4..P5
        ao_pool = es_attn.enter_context(tc.tile_pool(name="ao_p", bufs=1, side="left"))
        attn_oT = ao_pool.tile([128, DC, R], BF16, name="attn_oT")
        wp_pool = es_attn.enter_context(tc.tile_pool(name="wp_p", bufs=1, side="left"))
        wp_sb = wp_pool.tile([128, DC, D], BF16, name="w_wp")
        nc.sync.dma_start(
            wp_sb[:], io["wp"][:].rearrange("(c p) d -> p c d", p=128))
        hs5_pool = es_attn.enter_context(
            tc.tile_pool(name="hs5_p", bufs=1, side="left"))
        hst_pre = []
        for rt in range(2):
            t5 = hs5_pool.tile([128, D], F32, name=f"hst5_{rt}")
            nc.sync.dma_start(t5[:], hs[128 * rt:128 * (rt + 1), :])
            hst_pre.append(t5)
        with tc.tile_pool(name="kg_pool", bufs=1, side="left") as kgp, \
             tc.tile_pool(name="vg_pool", bufs=1, side="left") as vgp, \
             tc.tile_pool(name="at_sb", bufs=4, side="left") as asb, \
             tc.tile_pool(name="at_norm", bufs=2, side="left") as anorm, \
             tc.tile_pool(name="oTB_p", bufs=1, side="left") as obp, \
             tc.tile_pool(name="sc_ps", bufs=2, space="PSUM") as scps, \
             tc.tile_pool(name="oT_ps", bufs=4, space="PSUM") as otps:

            hps = (slice(0, 64), slice(64, 128))

            # ---- P4a: diagonal (own-kv) attention from SBUF-resident
            # kT/vaug, runs during the AllGathers; block b lands in q columns
            # [QB*b, QB*(b+1)) of the combined 512-wide partial ----
            oTB_sb = {}
            for hh in range(H // 2):
                h0, h1 = 2 * hh, 2 * hh + 1
                vss = (slice((HD + 1) * h0, (HD + 1) * (h0 + 1)),
                       slice((HD + 1) * h1, (HD + 1) * (h1 + 1)))
                obs = [obp.tile([HD + 1, R], BF16, tag=f"oTB{hh}_{j}",
                                name=f"oTBs_{hh}_{j}") for j in range(2)]
                for b in range(NBLK):
                    qs = slice(QB * b, QB * (b + 1))
                    oTBs = [otps.tile([HD + 1, QB], F32, tag="oT",
                                      name=f"oTB_{b}_{hh}_{j}")
                            for j in range(2)]
                    for i in range(2):
                        sl = 2 * b + i
                        sc = scps.tile([128, 2, 512], F32, tag="sc",
                                       name=f"scB_{b}_{hh}_{i}")
                        scv = sc[:, :, 0:QB]
                        for j in range(2):
                            nc.tensor.matmul(
                                sc[:, j, 0:QB],
                                kT[hps[j], hh, KTW * sl:KTW * (sl + 1)],
                                qT[hps[j], hh, qs],
                                start=True, stop=True)
                        m_sb = maskA_sb if i == 0 else maskB_sb
                        nc.vector.tensor_add(
                            scv, scv,
                            m_sb.rearrange("p (a b) -> p a b", a=2))
                        ex = asb.tile([128, 2, QB], BF16, tag="ex",
                                      name=f"exB_{b}_{hh}_{i}")
                        nc.scalar.activation(ex[:], scv, func=AF.Exp,
                                             bias=biasB_sb[:, b, i:i + 1],
                                             scale=1.0)
                        for j in range(2):
                            nc.tensor.matmul(oTBs[j][:],
                                             vaug[:, sl, vss[j]],
                                             ex[:, j, :],
                                             start=(i == 0), stop=(i == 1))
                    for j in range(2):
                        nc.scalar.copy(obs[j][:, qs], oTBs[j][:])
                for j in range(2):
                    oTB_sb[(hh, j)] = obs[j]

            # kT/vaug end with the diagonal pass; the ex ring reuses their
            # SBUF space (WAR deps keep the reuse safe)
            es_kv.close()
            es_ex = contextlib.ExitStack()
            expool = es_ex.enter_context(
                tc.tile_pool(name="ex_p", bufs=1, side="right"))

            # ---- gathered k/v loads (blocked on the AllGathers); rank 0's
            # upper half is position 7, never attended -> not loaded ----
            kranks, vranks = [], []
            for r in range(4):
                kw = QB if r == 0 else R
                kr = kgp.tile([128, DC, kw], BF16, tag=f"kr{r}",
                              name=f"kr_{r}")
                nc.sync.dma_start(
                    kr[:], k_g[r, :, 0:kw].rearrange("(c p) q -> p c q",
                                                     p=128))
                kranks.append(kr)
                vf = 2 if r == 0 else 4
                vr = vgp.tile([128, vf, VW], BF16, tag=f"vr{r}",
                              name=f"vr_{r}")
                nc.sync.dma_start(
                    vr[:], v_g[r, 0:128 * vf, :].rearrange("(f p) w -> p f w",
                                                           p=128))
                vranks.append(vr)

            def ktile_ap(t, hp, hc):
                r, off = _gtile_src(t)
                return kranks[r][hp, hc, off:off + KTW]

            def vtile_ap(t, vs):
                r, off = _gtile_src(t)
                return vranks[r][:, off // 128, vs]

            # ---- P4b: software-pipelined 14-slot kv sweep, interleaved at
            # slot granularity: qk+exp "score" slots (need only the k gather)
            # run LAG head-pairs ahead of the attn@v slots (which wait on the
            # v gather). Slots 0..5 hit both q-blocks in one 512-wide matmul
            # (block0's depth 2p never exceeds 6); slots 6..13 can only ever
            # feed block1 (depth 14-2p >= 8), so they run 256-wide on its q
            # columns alone. Per-core biasA kills the causally out-of-range
            # remainder.
            LAG = 3
            NWIDE = 6  # slots needing both q-blocks

            def score_slot(hh, t):
                sc = scps.tile([128, 2, R], F32, tag="sc",
                               name=f"scA_{hh}_{t}")
                wq_cols = R if t < NWIDE else QB
                qs = slice(0, R) if t < NWIDE else slice(QB, R)
                for j in range(2):
                    nc.tensor.matmul(sc[:, j, 0:wq_cols],
                                     ktile_ap(t, hps[j], hh),
                                     qT[hps[j], hh, qs],
                                     start=True, stop=True)
                ex = expool.tile([128, 2, wq_cols], BF16,
                                 tag=f"ex{t}_{hh % LAG}",
                                 name=f"exA_{hh}_{t}")
                if t < NWIDE:
                    for half in range(2):
                        qh = slice(QB * half, QB * (half + 1))
                        nc.scalar.activation(
                            ex[:, :, qh], sc[:, :, qh], func=AF.Exp,
                            bias=biasA_sb[:, t, half:half + 1], scale=1.0)
                else:
                    nc.scalar.activation(ex[:], sc[:, :, 0:QB], func=AF.Exp,
                                         bias=biasA_sb[:, t, 1:2], scale=1.0)
                return ex

            def av_slot(hh, t, ex, oTs):
                vss = (slice((HD + 1) * 2 * hh, (HD + 1) * (2 * hh + 1)),
                       slice((HD + 1) * (2 * hh + 1), (HD + 1) * (2 * hh + 2)))
                os_ = slice(0, R) if t < NWIDE else slice(QB, R)
                for j in range(2):
                    nc.tensor.matmul(oTs[j][:, os_], vtile_ap(t, vss[j]),
                                     ex[:, j, :],
                                     start=(t == 0), stop=False,
                                     skip_group_check=True)

            def combine(hh, oTs):
                for j, h in enumerate((2 * hh, 2 * hh + 1)):
                    oT = oTs[j]
                    # fold the diagonal partial in on the PE (identity
                    # matmul closes the psum accumulation group), keeping
                    # the vector engine off the critical path
                    nc.tensor.matmul(oT[:], ident_sb[0:HD + 1, 0:HD + 1],
                                     oTB_sb[(hh, j)][:],
                                     start=False, stop=True,
                                     skip_group_check=True)
                    rec = anorm.tile([1, R], F32, tag="rec", name=f"rec_{h}")
                    nc.vector.reciprocal(rec[:], oT[HD:HD + 1, :])
                    rb = anorm.tile([64, R], F32, tag="rb", name=f"rb_{h}")
                    nc.gpsimd.partition_broadcast(rb[:], rec[:])
                    if j == 0:
                        dst = attn_oT[0:HD, hh, :]
                        nc.vector.tensor_mul(dst, oT[0:HD, :], rb[:])
                        if apply_bv:
                            nc.vector.tensor_scalar_add(
                                dst, dst, bvh_sb[:, h:h + 1])
                    else:
                        tmpn = anorm.tile([64, R], BF16, tag="tmpn",
                                          name=f"tmpn_{h}")
                        nc.vector.tensor_mul(tmpn[:], oT[0:HD, :], rb[:])
                        if apply_bv:
                            nc.vector.tensor_scalar_add(
                                tmpn[:], tmpn[:], bvh_sb[:, h:h + 1])
                        nc.sync.dma_start(attn_oT[64:128, hh, :], tmpn[:])

            pend, live_oTs = {}, {}
            for step in range(H // 2 + LAG):
                if step < H // 2:
                    pend[step] = []
                if step >= LAG:
                    live_oTs[step - LAG] = [
                        otps.tile([HD + 1, R], F32, tag="oT",
                                  name=f"oT_{step - LAG}_{j}")
                        for j in range(2)]
                for t in range(NSLOT):
                    # av before score: the ex slot score(step, t) reuses
                    # (ring of LAG) must have its reader av emitted first
                    if step >= LAG:
                        av_slot(step - LAG, t, pend[step - LAG][t],
                                live_oTs[step - LAG])
                    if step < H // 2:
                        pend[step].append(score_slot(step, t))
                if step >= LAG:
                    combine(step - LAG, live_oTs.pop(step - LAG))
                    del pend[step - LAG]
            es_ex.close()

        # ================= P5+P6: out-proj + residual, fused with LN2 +
        # transpose per row-tile so the LN/transpose latency hides under the
        # next row-tile's projection chains =================================
        es_h = ctx.enter_context(contextlib.ExitStack())      # h_sb: P5..P8
        h_pool = es_h.enter_context(tc.tile_pool(name="h_p", bufs=1, side="right"))
        h_sb = h_pool.tile([128, 4, D], F32)
        es_mlp = ctx.enter_context(contextlib.ExitStack())    # h2T, gT, w2
        mlp_pool = es_mlp.enter_context(tc.tile_pool(name="mlp_p", bufs=1, side="right"))
        h2T = mlp_pool.tile([128, DC, R], BF16)
        gT = mlp_pool.tile([128, GC, R], BF16)
        w2_sb = mlp_pool.tile([128, GC, D], BF16)
        with tc.tile_pool(name="hs2", bufs=2, side="left") as hs2, \
             tc.tile_pool(name="p6", bufs=2, side="left") as p6, \
             tc.tile_pool(name="p6ps", bufs=4, space="PSUM") as p6ps, \
             tc.tile_pool(name="ps_wp", bufs=2, space="PSUM") as pps:
            wt = wp_sb
            for rt in range(4):
                if rt < 2:
                    hst = hst_pre[rt]
                else:
                    hst = hs2.tile([128, D], F32, tag="hst",
                                   name=f"hst2_{rt}")
                    nc.sync.dma_start(hst[:], hs[128 * rt:128 * (rt + 1), :])
                for cg in range(2):
                    ps = pps.tile([128, 512], F32, tag="ps",
                                  name=f"ps_wp_{rt}_{cg}")
                    nc.tensor.matmul(ps[:], ones_r[:],
                                     bpr_sb[:, 512 * cg:512 * (cg + 1)],
                                     start=True, stop=False)
                    for c in range(DC):
                        nc.tensor.matmul(
                            ps[:], attn_oT[:, c, 128 * rt:128 * (rt + 1)],
                            wt[:, c, 512 * cg:512 * (cg + 1)],
                            start=False, stop=(c == DC - 1))
                    nc.vector.tensor_add(
                        h_sb[:, rt, 512 * cg:512 * (cg + 1)],
                        ps[:], hst[:, 512 * cg:512 * (cg + 1)])
                h2 = p6.tile([128, D], BF16, tag="h2")
                layernorm(h2[:], h_sb[:, rt, :], p6, ln2_g, ln2_b)
                transpose_into(h2T, h2, rt, p6ps)
        es_attn.close()  # attn_oT + wp done

        # ================= P7: MLP up + gelu (w2 prefetch underneath) ========
        nc.sync.dma_start(
            w2_sb[:], io["w2"][:].rearrange("(g p) d -> p g d", p=128))
        with tc.tile_pool(name="w_w1", bufs=3, side="left") as wpl, \
             tc.tile_pool(name="ps_w1", bufs=2, space="PSUM") as pps:
            for gc in range(GC):
                wt = wpl.tile([128, DC, 128], BF16, tag="w1")
                nc.sync.dma_start(wt[:], io["w1"][gc])
                ps = pps.tile([128, R], F32, tag="ps", name=f"ps_w1_{gc}")
                for c in range(DC):
                    nc.tensor.matmul(ps[:], wt[:, c, :], h2T[:, c, :],
                                     start=(c == 0), stop=(c == DC - 1))
                nc.scalar.activation(gT[:, gc, :], ps[:], func=AF.Gelu,
                                     bias=b1l_sb[:, gc:gc + 1], scale=1.0)

        # ================= P8: MLP down, qt-major + bias + residual ==========
        with tc.tile_pool(name="o_sb", bufs=2, side="left") as osb, \
             tc.tile_pool(name="o_ps", bufs=2, space="PSUM") as pps:
            for qt in range(4):
                ps = pps.tile([128, 2, 512], F32, tag="ops", name=f"o_ps_{qt}")
                for cg in range(2):
                    nc.tensor.matmul(ps[:, cg, :], ones_r[:],
                                     b2r_sb[:, 512 * cg:512 * (cg + 1)],
                                     start=True, stop=False)
                for gc in range(GC):
                    for cg in range(2):
                        nc.tensor.matmul(
                            ps[:, cg, :],
                            gT[:, gc, 128 * qt:128 * (qt + 1)],
                            w2_sb[:, gc, 512 * cg:512 * (cg + 1)],
                            start=False, stop=(gc == GC - 1))
                ot = osb.tile([128, D], F32, tag="ot", name=f"ot_{qt}")
                nc.vector.tensor_add(ot[:],
                                     ps[:].rearrange("p a b -> p (a b)"),
                                     h_sb[:, qt, :])
                nc.sync.dma_start(out[128 * qt:128 * (qt + 1), :], ot[:])


# ---------------------------------------------------------------------------
# Host side
# ---------------------------------------------------------------------------

_CACHE = {}
LAST_RESULT = None  # BassKernelResults of the most recent run (for test.py)


def _get_program(key):
    if key not in _CACHE:
        _CACHE[key] = build_program(*key)
    return _CACHE[key]


def _colzero_bias(kpos):
    return np.where((kpos % JD) == (JD - 1), np.float32(NEG), np.float32(0.0))


def kernel(hidden_states, Wq, bq, Wk, bk, Wv, bv, Wp, bp,
           ln1_g, ln1_b, ln2_g, ln2_b, W1, b1, W2, b2):
    f32 = lambda a: np.ascontiguousarray(np.asarray(a, dtype=np.float32))
    hidden_states = f32(hidden_states)
    Wq, bq, Wk, bk, Wv, bv, Wp, bp = map(f32, (Wq, bq, Wk, bk, Wv, bv, Wp, bp))
    ln1_g, ln1_b, ln2_g, ln2_b = map(f32, (ln1_g, ln1_b, ln2_g, ln2_b))
    W1, b1, W2, b2 = map(f32, (W1, b1, W2, b2))

    apply_bv = bool(np.any(bv != 0.0))
    apply_ln1 = bool(np.any(ln1_g != 1.0) or np.any(ln1_b != 0.0))
    apply_ln2 = bool(np.any(ln2_g != 1.0) or np.any(ln2_b != 0.0))
    nc = _get_program((apply_bv, apply_ln1, apply_ln2))

    chunk_major = lambda v: np.ascontiguousarray(v.reshape(-1, 128).T)
    kp = np.arange(KTW)[:, None]
    iq = np.arange(KTW)[None, :]
    tri = np.where(kp <= iq, np.float32(0.0), np.float32(NEG))
    maskAB = np.zeros((2, 128, 2 * KTW), dtype=np.float32)
    maskAB[0, :, :KTW] = tri
    maskAB[1, :, :KTW] = NEG
    maskAB[1, :, KTW:] = tri

    import ml_dtypes
    bf = lambda a: np.ascontiguousarray(a.astype(ml_dtypes.bfloat16))
    w1x = np.ascontiguousarray(
        W1.reshape(DC, 128, GC, 128).transpose(2, 1, 0, 3))
    shared = dict(wq=bf(Wq), wk=bf(Wk), wv=bf(Wv), wp=bf(Wp), w1=bf(w1x),
                  w2=bf(W2),
                  bq8=chunk_major(bq * 0.125), bkl=chunk_major(bk),
                  bvh=np.ascontiguousarray(bv.reshape(H, HD).T),
                  b1l=chunk_major(b1), bpr=bf(bp.reshape(1, D)),
                  b2r=bf(b2.reshape(1, D)), ln1gb=np.stack([ln1_g, ln1_b]),
                  ln2gb=np.stack([ln2_g, ln2_b]), maskAB=maskAB,
                  ident=np.eye(128, dtype=ml_dtypes.bfloat16),
                  onesr=np.ones((1, 128), dtype=ml_dtypes.bfloat16))

    in_maps, row_map = [], []
    for core in range(NCORE):
        # cores 0-3: batch 0, positions (p, 7-p); cores 4-7: batch 1 mirror
        batch, p = core // 4, core % 4
        positions = (p, 7 - p)
        rows = [np.arange(QB * pb, QB * (pb + 1)) for pb in positions]
        row_map.append((batch, rows))
        depths = (2 * p, 14 - 2 * p)   # pass-A kv tiles needed per block

        biasA = np.empty((128, NSLOT, 2), dtype=np.float32)
        for t in range(NSLOT):
            cz = _colzero_bias(KTW * t + np.arange(KTW))
            for half in range(2):
                biasA[:, t, half] = cz if t < depths[half] else NEG
        biasB = np.zeros((128, NBLK, 2), dtype=np.float32)
        for b, pb in enumerate(positions):
            for i in range(2):
                biasB[:, b, i] = _colzero_bias(QB * pb + KTW * i
                                               + np.arange(KTW))

        m = dict(shared)
        m["hs"] = np.ascontiguousarray(
            np.concatenate([hidden_states[batch, rows[0], :],
                            hidden_states[batch, rows[1], :]]))
        m["biasA"] = np.ascontiguousarray(biasA)
        m["biasB"] = np.ascontiguousarray(biasB)
        in_maps.append(m)

    res = run_bass_kernel_spmd(nc, in_maps, core_ids=list(range(NCORE)))
    global LAST_RESULT
    LAST_RESULT = res

    out_full = np.empty((B, S, D), dtype=np.float32)
    for core in range(NCORE):
        batch, rows = row_map[core]
        o = res.results[core]["out"]
        out_full[batch, rows[0], :] = o[:QB]
        out_full[batch, rows[1], :] = o[QB:]
    return out_full


# revision 81
# speedup vs baseline: 1.0692x; 1.0692x over previous
"""Trainium2 Bass kernel for nn_Block_55336358643145 (dense transformer block).

Row-shards the 4096 (batch*seq) rows across 8 NeuronCores, 512 rows per core as
two 256-row blocks of the SAME batch (cores 0-3 carry batch 0 positions
(p, 7-p); cores 4-7 mirror for batch 1), so both the k and v AllGathers run as
two independent 4-core groups and both q-blocks of a core read the same
gathered kv stream. Per core: LN1 -> transpose -> Q/K/V (q,k head-transposed,
v row-layout with a per-head ones column so the softmax denominator falls out
of the attn@v matmul), chunked grouped AllGathers of k/v (k_a, v_a, k_b, v_b)
pipelined against production and consumption, diagonal (own-kv) attention from
SBUF-resident kT/vaug overlapped with the collectives, then a uniform 14-slot
kv sweep where each 128-row kv tile hits both q-blocks in one 512-wide matmul
(per-core bias columns kill the halves that are causally out of range:
exp == 0 exactly), attn@v, out-proj + residual, LN2, 4x MLP with exact-erf
Gelu and a qt-major down-proj against SBUF-resident W2 so the tail drains
fast. Host reassembles.
"""

import contextlib

import numpy as np

import concourse.bass as bass
import concourse.tile as tile
from concourse import bacc, mybir
from concourse.bass_utils import run_bass_kernel_spmd

F32 = mybir.dt.float32
F32R = mybir.dt.float32r
AF = mybir.ActivationFunctionType
ALU = mybir.AluOpType

B, S, D, H, HD, FF = 2, 2048, 1024, 16, 64, 4096
NCORE = 8
R = 512            # rows per core
QB = 256           # q rows per block
NBLK = 2           # blocks per core
KTW = 128          # kpos tile width
NSLOT = 14         # pass-A kv tile slots (uniform across cores)
LN_EPS = 1e-5
JD = 25            # joined dim for the column-zero mask
NEG = -1.0e30
DC = D // 128      # 8 d-chunks
GC = FF // 128     # 32 mlp hidden chunks
VW = H * (HD + 1)  # 1040: v with per-head ones column


BF16 = mybir.dt.bfloat16


def _gtile_src(t):
    """128-wide kv tile t of this batch -> (group rank, col/row offset)."""
    p = t // 2
    if p <= 3:
        return p, 128 * (t % 2)          # position p -> rank p, block0 half
    return 7 - p, 256 + 128 * (t % 2)    # position p -> rank 7-p, block1 half


def build_program(apply_bv, apply_ln1_gb, apply_ln2_gb):
    nc = bacc.Bacc("TRN2", target_bir_lowering=False, debug=False,
                   num_devices=NCORE)

    def inp(name, shape):
        return nc.dram_tensor(name, list(shape), F32, kind="ExternalInput").ap()

    def binp(name, shape):
        return nc.dram_tensor(name, list(shape), BF16,
                              kind="ExternalInput").ap()

    io = dict(
        hs=inp("hs", (R, D)),
        wq=binp("wq", (D, D)), wk=binp("wk", (D, D)),
        wv=binp("wv", (D, D)), wp=binp("wp", (D, D)),
        w1=binp("w1", (GC, 128, DC, 128)), w2=binp("w2", (FF, D)),
        bq8=inp("bq8", (128, DC)), bkl=inp("bkl", (128, DC)),
        bvh=inp("bvh", (HD, H)), b1l=inp("b1l", (128, GC)),
        bpr=binp("bpr", (1, D)), b2r=binp("b2r", (1, D)),
        ln1gb=inp("ln1gb", (2, D)), ln2gb=inp("ln2gb", (2, D)),
        biasA=inp("biasA", (128, NSLOT, 2)),
        biasB=inp("biasB", (128, NBLK, 2)),
        maskAB=inp("maskAB", (2, 128, 2 * KTW)),
        ident=binp("ident", (128, 128)),
        onesr=binp("onesr", (1, 128)),
        out=nc.dram_tensor("out", [R, D], F32, kind="ExternalOutput").ap(),
    )

    with tile.TileContext(nc) as tc:
        _build(tc, io, apply_bv, apply_ln1_gb, apply_ln2_gb)
    nc.compile()
    return nc


def _build(tc, io, apply_bv, apply_ln1_gb, apply_ln2_gb):
    nc = tc.nc
    hs, out = io["hs"], io["out"]

    with contextlib.ExitStack() as ctx:
        persist = ctx.enter_context(tc.tile_pool(name="persist", bufs=1, side="left"))
        dram = ctx.enter_context(tc.tile_pool(name="dram", bufs=1,
                                              space="DRAM"))

        # ---- P0-critical constants only; the rest load later ----------------
        eps_sb = persist.tile([128, 1], F32)
        nc.vector.memset(eps_sb[:], LN_EPS)
        ident_sb = persist.tile([128, 128], BF16)
        nc.sync.dma_start(ident_sb[:], io["ident"][:])

        # projection weight tiles; wk's load is issued here (first consumer is
        # P1), wv/wq loads are issued after P0 so the hs tiles win the queue
        es_w = ctx.enter_context(contextlib.ExitStack())      # wk/wv/wq: ..P3
        wqkv_pool = es_w.enter_context(
            tc.tile_pool(name="wqkv_p", bufs=1, side="left"))
        wqkv = {}
        for nm in ("wk", "wv", "wq"):
            wqkv[nm] = wqkv_pool.tile([128, DC, D], BF16, name=f"w_{nm}")

        def ln_gb_tiles(gb_inp, nm):
            g_sb = persist.tile([128, D], F32, name=f"g_{nm}")
            b_sb = persist.tile([128, D], F32, name=f"b_{nm}")
            g_row = persist.tile([1, D], F32, name=f"gr_{nm}")
            b_row = persist.tile([1, D], F32, name=f"br_{nm}")
            nc.sync.dma_start(g_row[:], gb_inp[0:1, :])
            nc.sync.dma_start(b_row[:], gb_inp[1:2, :])
            nc.gpsimd.partition_broadcast(g_sb[:], g_row[:])
            nc.gpsimd.partition_broadcast(b_sb[:], b_row[:])
            return g_sb, b_sb

        ln1_g = ln1_b = ln2_g = ln2_b = None
        if apply_ln1_gb:
            ln1_g, ln1_b = ln_gb_tiles(io["ln1gb"], "ln1")

        def layernorm(dst, src, pool, g_sb, b_sb):
            stats = pool.tile([128, 2, 6], F32, tag="ln_stats")
            sg = src.rearrange("p (g d) -> p g d", g=2)
            for g in range(2):
                nc.vector.bn_stats(out=stats[:, g, :], in_=sg[:, g, :])
            mv = pool.tile([128, 2], F32, tag="ln_mv")
            nc.vector.bn_aggr(out=mv[:], in_=stats[:])
            rstd = pool.tile([128, 1], F32, tag="ln_rstd")
            nc.scalar.activation(out=rstd[:], in_=mv[:, 1:2], func=AF.Sqrt,
                                 bias=eps_sb[:], scale=1.0)
            nc.vector.reciprocal(out=rstd[:], in_=rstd[:])
            nc.vector.tensor_scalar(out=dst, in0=src, scalar1=mv[:, 0:1],
                                    scalar2=rstd[:], op0=ALU.subtract,
                                    op1=ALU.mult)
            if g_sb is not None:
                nc.vector.tensor_mul(dst, dst, g_sb[:])
                nc.vector.tensor_add(dst, dst, b_sb[:])

        def transpose_into(dstT, src_tile, rt, tp_pool):
            for c in range(DC):
                tp = tp_pool.tile([128, 128], BF16, tag="tp")
                nc.tensor.transpose(tp[:], src_tile[:, 128 * c:128 * (c + 1)],
                                    ident_sb[:])
                nc.scalar.copy(dstT[:, c, 128 * rt:128 * (rt + 1)], tp[:])

        es_x = ctx.enter_context(contextlib.ExitStack())      # xT: P0..P3
        es_q = ctx.enter_context(contextlib.ExitStack())      # qT: P3..P4
        xT_pool = es_x.enter_context(
            tc.tile_pool(name="xT_p", bufs=1, side="left"))
        qT_pool = es_q.enter_context(
            tc.tile_pool(name="qT_p", bufs=1, side="right"))
        xT = xT_pool.tile([128, DC, R], BF16)
        qT = qT_pool.tile([128, DC, R], BF16)

        nc.sync.dma_start(
            wqkv["wk"][:], io["wk"][:].rearrange("(c p) d -> p c d", p=128))

        # ================= P0: load + LN1 + transpose ========================
        with tc.tile_pool(name="p0", bufs=2, side="left") as p0, \
             tc.tile_pool(name="p0ps", bufs=4, space="PSUM") as p0ps:
            for rt in range(4):
                hst = p0.tile([128, D], F32, tag="hst")
                nc.sync.dma_start(hst[:], hs[128 * rt:128 * (rt + 1), :])
                xln = p0.tile([128, D], BF16, tag="xln")
                layernorm(xln[:], hst[:], p0, ln1_g, ln1_b)
                transpose_into(xT, xln, rt, p0ps)
        nc.sync.dma_start(
            wqkv["wv"][:], io["wv"][:].rearrange("(c p) d -> p c d", p=128))

        # ---- remaining small constants (off the P0 critical path) ----------
        bq8_sb = persist.tile([128, DC], F32)
        nc.sync.dma_start(bq8_sb[:], io["bq8"][:])
        bkl_sb = persist.tile([128, DC], F32)
        nc.sync.dma_start(bkl_sb[:], io["bkl"][:])
        b1l_sb = persist.tile([128, GC], F32)
        nc.sync.dma_start(b1l_sb[:], io["b1l"][:])
        ones_r = persist.tile([1, 128], BF16)
        nc.sync.dma_start(ones_r[:], io["onesr"][:])
        bpr_sb = persist.tile([1, D], BF16)
        nc.sync.dma_start(bpr_sb[:], io["bpr"][:])
        b2r_sb = persist.tile([1, D], BF16)
        nc.sync.dma_start(b2r_sb[:], io["b2r"][:])
        biasA_sb = persist.tile([128, NSLOT, 2], F32)
        nc.sync.dma_start(biasA_sb[:], io["biasA"][:])
        biasB_sb = persist.tile([128, NBLK, 2], F32)
        nc.sync.dma_start(biasB_sb[:], io["biasB"][:])
        maskA_sb = persist.tile([128, 2, 2 * KTW], F32)
        for j in range(2):
            nc.sync.dma_start(maskA_sb[:, j, :], io["maskAB"][0])
        maskA_sb = maskA_sb[:].rearrange("p a b -> p (a b)")
        maskB_sb = persist.tile([128, 2, 2 * KTW], F32)
        for j in range(2):
            nc.sync.dma_start(maskB_sb[:, j, :], io["maskAB"][1])
        maskB_sb = maskB_sb[:].rearrange("p a b -> p (a b)")
        if apply_bv:
            bvh_sb = persist.tile([HD, H], F32)
            nc.sync.dma_start(bvh_sb[:], io["bvh"][:])
        ln2_g = ln2_b = None
        if apply_ln2_gb:
            ln2_g, ln2_b = ln_gb_tiles(io["ln2gb"], "ln2")

        def proj_headT(dstT, wt, bias_sb, scale, nm):
            """dstT[:, oc, :] = ((x @ w) * scale + bias)^T rows 128oc..+128."""
            with tc.tile_pool(name=f"ps_{nm}", bufs=2, space="PSUM") as pps:
                for oc in range(DC):
                    ps = pps.tile([128, R], F32, tag="ps", name=f"ps_{nm}_{oc}")
                    for c in range(DC):
                        nc.tensor.matmul(
                            ps[:], wt[:, c, 128 * oc:128 * (oc + 1)],
                            xT[:, c, :], start=(c == 0), stop=(c == DC - 1))
                    nc.scalar.activation(dstT[:, oc, :], ps[:],
                                         func=AF.Identity,
                                         bias=bias_sb[:, oc:oc + 1],
                                         scale=scale)

        # DRAM bounce + gathered buffers for the grouped (4-way) k/v AllGathers
        k_loc = dram.tile([D, R], BF16)
        v_loc = dram.tile([R, VW], BF16)
        k_g = dram.tile([4, D, R], BF16)
        v_g = dram.tile([4, R, VW], BF16)
        GROUPS = [[0, 1, 2, 3], [4, 5, 6, 7]]

        es_kv = ctx.enter_context(contextlib.ExitStack())     # kT/vaug: ..P4a
        kT_pool = es_kv.enter_context(
            tc.tile_pool(name="kT_p", bufs=1, side="right"))
        vaug_pool = es_kv.enter_context(
            tc.tile_pool(name="vaug_p", bufs=1, side="right"))
        kT = kT_pool.tile([128, DC, R], BF16)
        vaug = vaug_pool.tile([128, 4, VW], BF16)

        # ================= P1: kT; store + AllGather(k) ======================
        proj_headT(kT, wqkv["wk"], bkl_sb, 1.0, "wk")
        nc.sync.dma_start(k_loc[:].rearrange("(c p) q -> p c q", p=128),
                          kT[:])
        nc.gpsimd.collective_compute(
            "AllGather", ALU.bypass, replica_groups=GROUPS,
            ins=[k_loc.opt()], outs=[k_g.opt()])
        # wq load deliberately after the k store + AllGather trigger: it is
        # not needed until P3 and would delay the collective on the DMA queue
        nc.sync.dma_start(
            wqkv["wq"][:], io["wq"][:].rearrange("(c p) d -> p c d", p=128))

        # ================= P2: v rows + ones cols; AllGather(v) ==============
        with tc.tile_pool(name="ps_wv", bufs=2, space="PSUM") as pps:
            # per-head softmax-denominator ones column
            nc.gpsimd.memset(
                vaug[:].rearrange("p f (h e) -> p f h e", e=HD + 1)
                [:, :, :, HD:HD + 1], 1.0)
            wt = wqkv["wv"]
            for pt in range(4):
                for cg in range(2):
                    ps = pps.tile([128, 512], F32, tag="ps",
                                  name=f"ps_wv_{pt}_{cg}")
                    for c in range(DC):
                        nc.tensor.matmul(
                            ps[:], xT[:, c, 128 * pt:128 * (pt + 1)],
                            wt[:, c, 512 * cg:512 * (cg + 1)],
                            start=(c == 0), stop=(c == DC - 1))
                    for hh in range(8):
                        h = 8 * cg + hh
                        nc.scalar.copy(
                            vaug[:, pt, (HD + 1) * h:(HD + 1) * h + HD],
                            ps[:, HD * hh:HD * (hh + 1)])
            nc.sync.dma_start(v_loc[:].rearrange("(f p) w -> p f w", p=128),
                              vaug[:])
            nc.gpsimd.collective_compute(
                "AllGather", ALU.bypass, replica_groups=GROUPS,
                ins=[v_loc.opt()], outs=[v_g.opt()])

        # ================= P3: qT (scaled by 1/8) ============================
        proj_headT(qT, wqkv["wq"], bq8_sb, 0.125, "wq")
        es_x.close()  # xT no longer needed
        es_w.close()  # wk/wv/wq no longer needed

        # ================= P4: attention =====================================
        es_attn = ctx.enter_context(contextlib.ExitStack())   # attn_oT: P4..P5
        ao_pool = es_attn.enter_context(tc.tile_pool(name="ao_p", bufs=1, side="left"))
        attn_oT = ao_pool.tile([128, DC, R], BF16, name="attn_oT")
        wp_pool = es_attn.enter_context(tc.tile_pool(name="wp_p", bufs=1, side="left"))
        wp_sb = wp_pool.tile([128, DC, D], BF16, name="w_wp")
        nc.sync.dma_start(
            wp_sb[:], io["wp"][:].rearrange("(c p) d -> p c d", p=128))
        hs5_pool = es_attn.enter_context(
            tc.tile_pool(name="hs5_p", bufs=1, side="left"))
        hst_pre = []
        for rt in range(2):
            t5 = hs5_pool.tile([128, D], F32, name=f"hst5_{rt}")
            nc.sync.dma_start(t5[:], hs[128 * rt:128 * (rt + 1), :])
            hst_pre.append(t5)
        with tc.tile_pool(name="kg_pool", bufs=1, side="left") as kgp, \
             tc.tile_pool(name="vg_pool", bufs=1, side="left") as vgp, \
             tc.tile_pool(name="at_sb", bufs=4, side="left") as asb, \
             tc.tile_pool(name="at_norm", bufs=2, side="left") as anorm, \
             tc.tile_pool(name="oTB_p", bufs=1, side="left") as obp, \
             tc.tile_pool(name="sc_ps", bufs=2, space="PSUM") as scps, \
             tc.tile_pool(name="oT_ps", bufs=4, space="PSUM") as otps:

            hps = (slice(0, 64), slice(64, 128))

            # ---- P4a: diagonal (own-kv) attention from SBUF-resident
            # kT/vaug, runs during the AllGathers; block b lands in q columns
            # [QB*b, QB*(b+1)) of the combined 512-wide partial ----
            oTB_sb = {}
            for hh in range(H // 2):
                h0, h1 = 2 * hh, 2 * hh + 1
                vss = (slice((HD + 1) * h0, (HD + 1) * (h0 + 1)),
                       slice((HD + 1) * h1, (HD + 1) * (h1 + 1)))
                obs = [obp.tile([HD + 1, R], BF16, tag=f"oTB{hh}_{j}",
                                name=f"oTBs_{hh}_{j}") for j in range(2)]
                for b in range(NBLK):
                    qs = slice(QB * b, QB * (b + 1))
                    oTBs = [otps.tile([HD + 1, QB], F32, tag="oT",
                                      name=f"oTB_{b}_{hh}_{j}")
                            for j in range(2)]
                    for i in range(2):
                        sl = 2 * b + i
                        sc = scps.tile([128, 2, 512], F32, tag="sc",
                                       name=f"scB_{b}_{hh}_{i}")
                        scv = sc[:, :, 0:QB]
                        for j in range(2):
                            nc.tensor.matmul(
                                sc[:, j, 0:QB],
                                kT[hps[j], hh, KTW * sl:KTW * (sl + 1)],
                                qT[hps[j], hh, qs],
                                start=True, stop=True)
                        m_sb = maskA_sb if i == 0 else maskB_sb
                        nc.vector.tensor_add(
                            scv, scv,
                            m_sb.rearrange("p (a b) -> p a b", a=2))
                        ex = asb.tile([128, 2, QB], BF16, tag="ex",
                                      name=f"exB_{b}_{hh}_{i}")
                        nc.scalar.activation(ex[:], scv, func=AF.Exp,
                                             bias=biasB_sb[:, b, i:i + 1],
                                             scale=1.0)
                        for j in range(2):
                            nc.tensor.matmul(oTBs[j][:],
                                             vaug[:, sl, vss[j]],
                                             ex[:, j, :],
                                             start=(i == 0), stop=(i == 1))
                    for j in range(2):
                        nc.scalar.copy(obs[j][:, qs], oTBs[j][:])
                for j in range(2):
                    oTB_sb[(hh, j)] = obs[j]

            # kT/vaug end with the diagonal pass; the ex ring reuses their
            # SBUF space (WAR deps keep the reuse safe)
            es_kv.close()
            es_ex = contextlib.ExitStack()
            expool = es_ex.enter_context(
                tc.tile_pool(name="ex_p", bufs=1, side="right"))

            # ---- gathered k/v loads (blocked on the AllGathers); rank 0's
            # upper half is position 7, never attended -> not loaded ----
            kranks, vranks = [], []
            for r in range(4):
                kw = QB if r == 0 else R
                kr = kgp.tile([128, DC, kw], BF16, tag=f"kr{r}",
                              name=f"kr_{r}")
                nc.sync.dma_start(
                    kr[:], k_g[r, :, 0:kw].rearrange("(c p) q -> p c q",
                                                     p=128))
                kranks.append(kr)
                vf = 2 if r == 0 else 4
                vr = vgp.tile([128, vf, VW], BF16, tag=f"vr{r}",
                              name=f"vr_{r}")
                nc.sync.dma_start(
                    vr[:], v_g[r, 0:128 * vf, :].rearrange("(f p) w -> p f w",
                                                           p=128))
                vranks.append(vr)

            def ktile_ap(t, hp, hc):
                r, off = _gtile_src(t)
                return kranks[r][hp, hc, off:off + KTW]

            def vtile_ap(t, vs):
                r, off = _gtile_src(t)
                return vranks[r][:, off // 128, vs]

            # ---- P4b: software-pipelined 14-slot kv sweep, interleaved at
            # slot granularity: qk+exp "score" slots (need only the k gather)
            # run LAG head-pairs ahead of the attn@v slots (which wait on the
            # v gather). Slots 0..5 hit both q-blocks in one 512-wide matmul
            # (block0's depth 2p never exceeds 6); slots 6..13 can only ever
            # feed block1 (depth 14-2p >= 8), so they run 256-wide on its q
            # columns alone. Per-core biasA kills the causally out-of-range
            # remainder.
            LAG = 3
            NWIDE = 6  # slots needing both q-blocks

            def score_slot(hh, t):
                sc = scps.tile([128, 2, R], F32, tag="sc",
                               name=f"scA_{hh}_{t}")
                wq_cols = R if t < NWIDE else QB
                qs = slice(0, R) if t < NWIDE else slice(QB, R)
                for j in range(2):
                    nc.tensor.matmul(sc[:, j, 0:wq_cols],
                                     ktile_ap(t, hps[j], hh),
                                     qT[hps[j], hh, qs],
                                     start=True, stop=True)
                ex = expool.tile([128, 2, wq_cols], BF16,
                                 tag=f"ex{t}_{hh % LAG}",
                                 name=f"exA_{hh}_{t}")
                if t < NWIDE:
                    for half in range(2):
                        qh = slice(QB * half, QB * (half + 1))
                        nc.scalar.activation(
                            ex[:, :, qh], sc[:, :, qh], func=AF.Exp,
                            bias=biasA_sb[:, t, half:half + 1], scale=1.0)
                else:
                    nc.scalar.activation(ex[:], sc[:, :, 0:QB], func=AF.Exp,
                                         bias=biasA_sb[:, t, 1:2], scale=1.0)
                return ex

            def av_slot(hh, t, ex, oTs):
                vss = (slice((HD + 1) * 2 * hh, (HD + 1) * (2 * hh + 1)),
                       slice((HD + 1) * (2 * hh + 1), (HD + 1) * (2 * hh + 2)))
                os_ = slice(0, R) if t < NWIDE else slice(QB, R)
                for j in range(2):
                    nc.tensor.matmul(oTs[j][:, os_], vtile_ap(t, vss[j]),
                                     ex[:, j, :],
                                     start=(t == 0), stop=False,
                                     skip_group_check=True)

            def combine(hh, oTs):
                for j, h in enumerate((2 * hh, 2 * hh + 1)):
                    oT = oTs[j]
                    # fold the diagonal partial in on the PE (identity
                    # matmul closes the psum accumulation group), keeping
                    # the vector engine off the critical path
                    nc.tensor.matmul(oT[:], ident_sb[0:HD + 1, 0:HD + 1],
                                     oTB_sb[(hh, j)][:],
                                     start=False, stop=True,
                                     skip_group_check=True)
                    rec = anorm.tile([1, R], F32, tag="rec", name=f"rec_{h}")
                    nc.vector.reciprocal(rec[:], oT[HD:HD + 1, :])
                    rb = anorm.tile([64, R], F32, tag="rb", name=f"rb_{h}")
                    nc.gpsimd.partition_broadcast(rb[:], rec[:])
                    if j == 0:
                        dst = attn_oT[0:HD, hh, :]
                        nc.vector.tensor_mul(dst, oT[0:HD, :], rb[:])
                        if apply_bv:
                            nc.vector.tensor_scalar_add(
                                dst, dst, bvh_sb[:, h:h + 1])
                    else:
                        tmpn = anorm.tile([64, R], BF16, tag="tmpn",
                                          name=f"tmpn_{h}")
                        nc.vector.tensor_mul(tmpn[:], oT[0:HD, :], rb[:])
                        if apply_bv:
                            nc.vector.tensor_scalar_add(
                                tmpn[:], tmpn[:], bvh_sb[:, h:h + 1])
                        nc.sync.dma_start(attn_oT[64:128, hh, :], tmpn[:])

            pend, live_oTs = {}, {}
            for step in range(H // 2 + LAG):
                if step < H // 2:
                    pend[step] = []
                if step >= LAG:
                    live_oTs[step - LAG] = [
                        otps.tile([HD + 1, R], F32, tag="oT",
                                  name=f"oT_{step - LAG}_{j}")
                        for j in range(2)]
                for t in range(NSLOT):
                    # av before score: the ex slot score(step, t) reuses
                    # (ring of LAG) must have its reader av emitted first
                    if step >= LAG:
                        av_slot(step - LAG, t, pend[step - LAG][t],
                                live_oTs[step - LAG])
                    if step < H // 2:
                        pend[step].append(score_slot(step, t))
                if step >= LAG:
                    combine(step - LAG, live_oTs.pop(step - LAG))
                    del pend[step - LAG]
            es_ex.close()

        # ================= P5+P6: out-proj + residual, fused with LN2 +
        # transpose per row-tile so the LN/transpose latency hides under the
        # next row-tile's projection chains =================================
        es_h = ctx.enter_context(contextlib.ExitStack())      # h_sb: P5..P8
        h_pool = es_h.enter_context(tc.tile_pool(name="h_p", bufs=1, side="right"))
        h_sb = h_pool.tile([128, 4, D], F32)
        es_mlp = ctx.enter_context(contextlib.ExitStack())    # h2T, gT, w2
        mlp_pool = es_mlp.enter_context(tc.tile_pool(name="mlp_p", bufs=1, side="right"))
        h2T = mlp_pool.tile([128, DC, R], BF16)
        gT = mlp_pool.tile([128, GC, R], BF16)
        w2_sb = mlp_pool.tile([128, GC, D], BF16)
        with tc.tile_pool(name="hs2", bufs=2, side="left") as hs2, \
             tc.tile_pool(name="p6", bufs=2, side="left") as p6, \
             tc.tile_pool(name="p6ps", bufs=4, space="PSUM") as p6ps, \
             tc.tile_pool(name="ps_wp", bufs=2, space="PSUM") as pps:
            wt = wp_sb
            for rt in range(4):
                if rt < 2:
                    hst = hst_pre[rt]
                else:
                    hst = hs2.tile([128, D], F32, tag="hst",
                                   name=f"hst2_{rt}")
                    nc.sync.dma_start(hst[:], hs[128 * rt:128 * (rt + 1), :])
                for cg in range(2):
                    ps = pps.tile([128, 512], F32, tag="ps",
                                  name=f"ps_wp_{rt}_{cg}")
                    nc.tensor.matmul(ps[:], ones_r[:],
                                     bpr_sb[:, 512 * cg:512 * (cg + 1)],
                                     start=True, stop=False)
                    for c in range(DC):
                        nc.tensor.matmul(
                            ps[:], attn_oT[:, c, 128 * rt:128 * (rt + 1)],
                            wt[:, c, 512 * cg:512 * (cg + 1)],
                            start=False, stop=(c == DC - 1))
                    nc.vector.tensor_add(
                        h_sb[:, rt, 512 * cg:512 * (cg + 1)],
                        ps[:], hst[:, 512 * cg:512 * (cg + 1)])
                h2 = p6.tile([128, D], BF16, tag="h2")
                layernorm(h2[:], h_sb[:, rt, :], p6, ln2_g, ln2_b)
                transpose_into(h2T, h2, rt, p6ps)
        es_attn.close()  # attn_oT + wp done

        # ================= P7: MLP up + gelu (w2 prefetch underneath) ========
        nc.sync.dma_start(
            w2_sb[:], io["w2"][:].rearrange("(g p) d -> p g d", p=128))
        with tc.tile_pool(name="w_w1", bufs=3, side="left") as wpl, \
             tc.tile_pool(name="ps_w1", bufs=2, space="PSUM") as pps:
            for gc in range(GC):
                wt = wpl.tile([128, DC, 128], BF16, tag="w1")
                nc.sync.dma_start(wt[:], io["w1"][gc])
                ps = pps.tile([128, R], F32, tag="ps", name=f"ps_w1_{gc}")
                for c in range(DC):
                    nc.tensor.matmul(ps[:], wt[:, c, :], h2T[:, c, :],
                                     start=(c == 0), stop=(c == DC - 1))
                nc.scalar.activation(gT[:, gc, :], ps[:], func=AF.Gelu,
                                     bias=b1l_sb[:, gc:gc + 1], scale=1.0)

        # ================= P8: MLP down, qt-major + bias + residual ==========
        with tc.tile_pool(name="o_sb", bufs=2, side="left") as osb, \
             tc.tile_pool(name="o_ps", bufs=2, space="PSUM") as pps:
            for qt in range(4):
                ps = pps.tile([128, 2, 512], F32, tag="ops", name=f"o_ps_{qt}")
                for cg in range(2):
                    nc.tensor.matmul(ps[:, cg, :], ones_r[:],
                                     b2r_sb[:, 512 * cg:512 * (cg + 1)],
                                     start=True, stop=False)
                for gc in range(GC):
                    for cg in range(2):
                        nc.tensor.matmul(
                            ps[:, cg, :],
                            gT[:, gc, 128 * qt:128 * (qt + 1)],
                            w2_sb[:, gc, 512 * cg:512 * (cg + 1)],
                            start=False, stop=(gc == GC - 1))
                ot = osb.tile([128, D], F32, tag="ot", name=f"ot_{qt}")
                nc.vector.tensor_add(ot[:],
                                     ps[:].rearrange("p a b -> p (a b)"),
                                     h_sb[:, qt, :])
                nc.sync.dma_start(out[128 * qt:128 * (qt + 1), :], ot[:])


# ---------------------------------------------------------------------------
# Host side
# ---------------------------------------------------------------------------

_CACHE = {}
LAST_RESULT = None  # BassKernelResults of the most recent run (for test.py)


def _get_program(key):
    if key not in _CACHE:
        _CACHE[key] = build_program(*key)
    return _CACHE[key]


def _colzero_bias(kpos):
    return np.where((kpos % JD) == (JD - 1), np.float32(NEG), np.float32(0.0))


def kernel(hidden_states, Wq, bq, Wk, bk, Wv, bv, Wp, bp,
           ln1_g, ln1_b, ln2_g, ln2_b, W1, b1, W2, b2):
    f32 = lambda a: np.ascontiguousarray(np.asarray(a, dtype=np.float32))
    hidden_states = f32(hidden_states)
    Wq, bq, Wk, bk, Wv, bv, Wp, bp = map(f32, (Wq, bq, Wk, bk, Wv, bv, Wp, bp))
    ln1_g, ln1_b, ln2_g, ln2_b = map(f32, (ln1_g, ln1_b, ln2_g, ln2_b))
    W1, b1, W2, b2 = map(f32, (W1, b1, W2, b2))

    apply_bv = bool(np.any(bv != 0.0))
    apply_ln1 = bool(np.any(ln1_g != 1.0) or np.any(ln1_b != 0.0))
    apply_ln2 = bool(np.any(ln2_g != 1.0) or np.any(ln2_b != 0.0))
    nc = _get_program((apply_bv, apply_ln1, apply_ln2))

    chunk_major = lambda v: np.ascontiguousarray(v.reshape(-1, 128).T)
    kp = np.arange(KTW)[:, None]
    iq = np.arange(KTW)[None, :]
    tri = np.where(kp <= iq, np.float32(0.0), np.float32(NEG))
    maskAB = np.zeros((2, 128, 2 * KTW), dtype=np.float32)
    maskAB[0, :, :KTW] = tri
    maskAB[1, :, :KTW] = NEG
    maskAB[1, :, KTW:] = tri

    import ml_dtypes
    bf = lambda a: np.ascontiguousarray(a.astype(ml_dtypes.bfloat16))
    w1x = np.ascontiguousarray(
        W1.reshape(DC, 128, GC, 128).transpose(2, 1, 0, 3))
    shared = dict(wq=bf(Wq), wk=bf(Wk), wv=bf(Wv), wp=bf(Wp), w1=bf(w1x),
                  w2=bf(W2),
                  bq8=chunk_major(bq * 0.125), bkl=chunk_major(bk),
                  bvh=np.ascontiguousarray(bv.reshape(H, HD).T),
                  b1l=chunk_major(b1), bpr=bf(bp.reshape(1, D)),
                  b2r=bf(b2.reshape(1, D)), ln1gb=np.stack([ln1_g, ln1_b]),
                  ln2gb=np.stack([ln2_g, ln2_b]), maskAB=maskAB,
                  ident=np.eye(128, dtype=ml_dtypes.bfloat16),
                  onesr=np.ones((1, 128), dtype=ml_dtypes.bfloat16))

    in_maps, row_map = [], []
    for core in range(NCORE):
        # cores 0-3: batch 0, positions (p, 7-p); cores 4-7: batch 1 mirror
        batch, p = core // 4, core % 4
        positions = (p, 7 - p)
        rows = [np.arange(QB * pb, QB * (pb + 1)) for pb in positions]
        row_map.append((batch, rows))
        depths = (2 * p, 14 - 2 * p)   # pass-A kv tiles needed per block

        biasA = np.empty((128, NSLOT, 2), dtype=np.float32)
        for t in range(NSLOT):
            cz = _colzero_bias(KTW * t + np.arange(KTW))
            for half in range(2):
                biasA[:, t, half] = cz if t < depths[half] else NEG
        biasB = np.zeros((128, NBLK, 2), dtype=np.float32)
        for b, pb in enumerate(positions):
            for i in range(2):
                biasB[:, b, i] = _colzero_bias(QB * pb + KTW * i
                                               + np.arange(KTW))

        m = dict(shared)
        m["hs"] = np.ascontiguousarray(
            np.concatenate([hidden_states[batch, rows[0], :],
                            hidden_states[batch, rows[1], :]]))
        m["biasA"] = np.ascontiguousarray(biasA)
        m["biasB"] = np.ascontiguousarray(biasB)
        in_maps.append(m)

    res = run_bass_kernel_spmd(nc, in_maps, core_ids=list(range(NCORE)))
    global LAST_RESULT
    LAST_RESULT = res

    out_full = np.empty((B, S, D), dtype=np.float32)
    for core in range(NCORE):
        batch, rows = row_map[core]
        o = res.results[core]["out"]
        out_full[batch, rows[0], :] = o[:QB]
        out_full[batch, rows[1], :] = o[QB:]
    return out_full


# revision 84
# speedup vs baseline: 1.0749x; 1.0053x over previous
"""Trainium2 Bass kernel for nn_Block_55336358643145 (dense transformer block).

Row-shards the 4096 (batch*seq) rows across 8 NeuronCores, 512 rows per core as
two 256-row blocks of the SAME batch (cores 0-3 carry batch 0 positions
(p, 7-p); cores 4-7 mirror for batch 1), so both the k and v AllGathers run as
two independent 4-core groups and both q-blocks of a core read the same
gathered kv stream. Per core: LN1 -> transpose -> Q/K/V (q,k head-transposed,
v row-layout with a per-head ones column so the softmax denominator falls out
of the attn@v matmul), chunked grouped AllGathers of k/v (k_a, v_a, k_b, v_b)
pipelined against production and consumption, diagonal (own-kv) attention from
SBUF-resident kT/vaug overlapped with the collectives, then a uniform 14-slot
kv sweep where each 128-row kv tile hits both q-blocks in one 512-wide matmul
(per-core bias columns kill the halves that are causally out of range:
exp == 0 exactly), attn@v, out-proj + residual, LN2, 4x MLP with exact-erf
Gelu and a qt-major down-proj against SBUF-resident W2 so the tail drains
fast. Host reassembles.
"""

import contextlib

import numpy as np

import concourse.bass as bass
import concourse.tile as tile
from concourse import bacc, mybir
from concourse.bass_utils import run_bass_kernel_spmd

F32 = mybir.dt.float32
F32R = mybir.dt.float32r
AF = mybir.ActivationFunctionType
ALU = mybir.AluOpType

B, S, D, H, HD, FF = 2, 2048, 1024, 16, 64, 4096
NCORE = 8
R = 512            # rows per core
QB = 256           # q rows per block
NBLK = 2           # blocks per core
KTW = 128          # kpos tile width
NSLOT = 14         # pass-A kv tile slots (uniform across cores)
LN_EPS = 1e-5
JD = 25            # joined dim for the column-zero mask
NEG = -1.0e30
DC = D // 128      # 8 d-chunks
GC = FF // 128     # 32 mlp hidden chunks
VW = H * (HD + 1)  # 1040: v with per-head ones column


BF16 = mybir.dt.bfloat16


def _gtile_src(t):
    """128-wide kv tile t of this batch -> (group rank, col/row offset)."""
    p = t // 2
    if p <= 3:
        return p, 128 * (t % 2)          # position p -> rank p, block0 half
    return 7 - p, 256 + 128 * (t % 2)    # position p -> rank 7-p, block1 half


def build_program(apply_bv, apply_ln1_gb, apply_ln2_gb):
    nc = bacc.Bacc("TRN2", target_bir_lowering=False, debug=False,
                   num_devices=NCORE)

    def inp(name, shape):
        return nc.dram_tensor(name, list(shape), F32, kind="ExternalInput").ap()

    def binp(name, shape):
        return nc.dram_tensor(name, list(shape), BF16,
                              kind="ExternalInput").ap()

    io = dict(
        hs=inp("hs", (R, D)),
        wq=binp("wq", (D, D)), wk=binp("wk", (D, D)),
        wv=binp("wv", (D, D)), wp=binp("wp", (D, D)),
        w1=binp("w1", (GC, 128, DC, 128)), w2=binp("w2", (FF, D)),
        bq8=inp("bq8", (128, DC)), bkl=inp("bkl", (128, DC)),
        bvh=inp("bvh", (HD, H)), b1l=inp("b1l", (128, GC)),
        bpr=binp("bpr", (1, D)), b2r=binp("b2r", (1, D)),
        ln1gb=inp("ln1gb", (2, D)), ln2gb=inp("ln2gb", (2, D)),
        biasA=inp("biasA", (128, NSLOT, 2)),
        biasB=inp("biasB", (128, NBLK, 2)),
        maskAB=inp("maskAB", (2, 128, 2 * KTW)),
        ident=binp("ident", (128, 128)),
        onesr=binp("onesr", (1, 128)),
        out=nc.dram_tensor("out", [R, D], F32, kind="ExternalOutput").ap(),
    )

    with tile.TileContext(nc) as tc:
        _build(tc, io, apply_bv, apply_ln1_gb, apply_ln2_gb)
    nc.compile()
    return nc


def _build(tc, io, apply_bv, apply_ln1_gb, apply_ln2_gb):
    nc = tc.nc
    hs, out = io["hs"], io["out"]

    with contextlib.ExitStack() as ctx:
        persist = ctx.enter_context(tc.tile_pool(name="persist", bufs=1, side="left"))
        dram = ctx.enter_context(tc.tile_pool(name="dram", bufs=1,
                                              space="DRAM"))

        # ---- P0-critical constants only; the rest load later ----------------
        eps_sb = persist.tile([128, 1], F32)
        nc.vector.memset(eps_sb[:], LN_EPS)
        ident_sb = persist.tile([128, 128], BF16)
        nc.sync.dma_start(ident_sb[:], io["ident"][:])

        # projection weight tiles; wk's load is issued here (first consumer is
        # P1), wv/wq loads are issued after P0 so the hs tiles win the queue
        es_w = ctx.enter_context(contextlib.ExitStack())      # wk/wv/wq: ..P3
        wqkv_pool = es_w.enter_context(
            tc.tile_pool(name="wqkv_p", bufs=1, side="left"))
        wqkv = {}
        for nm in ("wk", "wv", "wq"):
            wqkv[nm] = wqkv_pool.tile([128, DC, D], BF16, name=f"w_{nm}")

        def ln_gb_tiles(gb_inp, nm):
            g_sb = persist.tile([128, D], F32, name=f"g_{nm}")
            b_sb = persist.tile([128, D], F32, name=f"b_{nm}")
            g_row = persist.tile([1, D], F32, name=f"gr_{nm}")
            b_row = persist.tile([1, D], F32, name=f"br_{nm}")
            nc.sync.dma_start(g_row[:], gb_inp[0:1, :])
            nc.sync.dma_start(b_row[:], gb_inp[1:2, :])
            nc.gpsimd.partition_broadcast(g_sb[:], g_row[:])
            nc.gpsimd.partition_broadcast(b_sb[:], b_row[:])
            return g_sb, b_sb

        ln1_g = ln1_b = ln2_g = ln2_b = None
        if apply_ln1_gb:
            ln1_g, ln1_b = ln_gb_tiles(io["ln1gb"], "ln1")

        def layernorm(dst, src, pool, g_sb, b_sb):
            stats = pool.tile([128, 2, 6], F32, tag="ln_stats")
            sg = src.rearrange("p (g d) -> p g d", g=2)
            for g in range(2):
                nc.vector.bn_stats(out=stats[:, g, :], in_=sg[:, g, :])
            mv = pool.tile([128, 2], F32, tag="ln_mv")
            nc.vector.bn_aggr(out=mv[:], in_=stats[:])
            rstd = pool.tile([128, 1], F32, tag="ln_rstd")
            nc.scalar.activation(out=rstd[:], in_=mv[:, 1:2], func=AF.Sqrt,
                                 bias=eps_sb[:], scale=1.0)
            nc.vector.reciprocal(out=rstd[:], in_=rstd[:])
            nc.vector.tensor_scalar(out=dst, in0=src, scalar1=mv[:, 0:1],
                                    scalar2=rstd[:], op0=ALU.subtract,
                                    op1=ALU.mult)
            if g_sb is not None:
                nc.vector.tensor_mul(dst, dst, g_sb[:])
                nc.vector.tensor_add(dst, dst, b_sb[:])

        def transpose_into(dstT, src_tile, rt, tp_pool):
            for c in range(DC):
                tp = tp_pool.tile([128, 128], BF16, tag="tp")
                nc.tensor.transpose(tp[:], src_tile[:, 128 * c:128 * (c + 1)],
                                    ident_sb[:])
                nc.scalar.copy(dstT[:, c, 128 * rt:128 * (rt + 1)], tp[:])

        es_x = ctx.enter_context(contextlib.ExitStack())      # xT: P0..P3
        es_q = ctx.enter_context(contextlib.ExitStack())      # qT: P3..P4
        xT_pool = es_x.enter_context(
            tc.tile_pool(name="xT_p", bufs=1, side="left"))
        qT_pool = es_q.enter_context(
            tc.tile_pool(name="qT_p", bufs=1, side="right"))
        xT = xT_pool.tile([128, DC, R], BF16)
        qT = qT_pool.tile([128, DC, R], BF16)

        nc.sync.dma_start(
            wqkv["wk"][:], io["wk"][:].rearrange("(c p) d -> p c d", p=128))

        # ================= P0: load + LN1 + transpose ========================
        with tc.tile_pool(name="p0", bufs=2, side="left") as p0, \
             tc.tile_pool(name="p0ps", bufs=4, space="PSUM") as p0ps:
            for rt in range(4):
                hst = p0.tile([128, D], F32, tag="hst")
                nc.sync.dma_start(hst[:], hs[128 * rt:128 * (rt + 1), :])
                xln = p0.tile([128, D], BF16, tag="xln")
                layernorm(xln[:], hst[:], p0, ln1_g, ln1_b)
                transpose_into(xT, xln, rt, p0ps)
        nc.sync.dma_start(
            wqkv["wv"][:], io["wv"][:].rearrange("(c p) d -> p c d", p=128))

        # ---- remaining small constants (off the P0 critical path) ----------
        bq8_sb = persist.tile([128, DC], F32)
        nc.sync.dma_start(bq8_sb[:], io["bq8"][:])
        bkl_sb = persist.tile([128, DC], F32)
        nc.sync.dma_start(bkl_sb[:], io["bkl"][:])
        b1l_sb = persist.tile([128, GC], F32)
        nc.sync.dma_start(b1l_sb[:], io["b1l"][:])
        ones_r = persist.tile([1, 128], BF16)
        nc.sync.dma_start(ones_r[:], io["onesr"][:])
        bpr_sb = persist.tile([1, D], BF16)
        nc.sync.dma_start(bpr_sb[:], io["bpr"][:])
        b2r_sb = persist.tile([1, D], BF16)
        nc.sync.dma_start(b2r_sb[:], io["b2r"][:])
        biasA_sb = persist.tile([128, NSLOT, 2], F32)
        nc.sync.dma_start(biasA_sb[:], io["biasA"][:])
        biasB_sb = persist.tile([128, NBLK, 2], F32)
        nc.sync.dma_start(biasB_sb[:], io["biasB"][:])
        maskA_sb = persist.tile([128, 2, 2 * KTW], F32)
        for j in range(2):
            nc.sync.dma_start(maskA_sb[:, j, :], io["maskAB"][0])
        maskA_sb = maskA_sb[:].rearrange("p a b -> p (a b)")
        maskB_sb = persist.tile([128, 2, 2 * KTW], F32)
        for j in range(2):
            nc.sync.dma_start(maskB_sb[:, j, :], io["maskAB"][1])
        maskB_sb = maskB_sb[:].rearrange("p a b -> p (a b)")
        if apply_bv:
            bvh_sb = persist.tile([HD, H], F32)
            nc.sync.dma_start(bvh_sb[:], io["bvh"][:])
        ln2_g = ln2_b = None
        if apply_ln2_gb:
            ln2_g, ln2_b = ln_gb_tiles(io["ln2gb"], "ln2")

        def proj_headT(dstT, wt, bias_sb, scale, nm):
            """dstT[:, oc, :] = ((x @ w) * scale + bias)^T rows 128oc..+128."""
            with tc.tile_pool(name=f"ps_{nm}", bufs=2, space="PSUM") as pps:
                for oc in range(DC):
                    ps = pps.tile([128, R], F32, tag="ps", name=f"ps_{nm}_{oc}")
                    for c in range(DC):
                        nc.tensor.matmul(
                            ps[:], wt[:, c, 128 * oc:128 * (oc + 1)],
                            xT[:, c, :], start=(c == 0), stop=(c == DC - 1))
                    nc.scalar.activation(dstT[:, oc, :], ps[:],
                                         func=AF.Identity,
                                         bias=bias_sb[:, oc:oc + 1],
                                         scale=scale)

        # DRAM bounce + gathered buffers for the grouped (4-way) k/v AllGathers
        k_loc = dram.tile([D, R], BF16)
        v_loc = dram.tile([R, VW], BF16)
        k_g = dram.tile([4, D, R], BF16)
        v_g = dram.tile([4, R, VW], BF16)
        GROUPS = [[0, 1, 2, 3], [4, 5, 6, 7]]

        es_kv = ctx.enter_context(contextlib.ExitStack())     # kT/vaug: ..P4a
        kT_pool = es_kv.enter_context(
            tc.tile_pool(name="kT_p", bufs=1, side="right"))
        vaug_pool = es_kv.enter_context(
            tc.tile_pool(name="vaug_p", bufs=1, side="right"))
        kT = kT_pool.tile([128, DC, R], BF16)
        vaug = vaug_pool.tile([128, 4, VW], BF16)

        # ================= P1: kT; store + AllGather(k) ======================
        proj_headT(kT, wqkv["wk"], bkl_sb, 1.0, "wk")
        nc.sync.dma_start(k_loc[:].rearrange("(c p) q -> p c q", p=128),
                          kT[:])
        nc.gpsimd.collective_compute(
            "AllGather", ALU.bypass, replica_groups=GROUPS,
            ins=[k_loc.opt()], outs=[k_g.opt()])
        # wq load deliberately after the k store + AllGather trigger: it is
        # not needed until P3 and would delay the collective on the DMA queue
        nc.sync.dma_start(
            wqkv["wq"][:], io["wq"][:].rearrange("(c p) d -> p c d", p=128))

        # ================= P2: v rows + ones cols; AllGather(v) ==============
        with tc.tile_pool(name="ps_wv", bufs=2, space="PSUM") as pps:
            # per-head softmax-denominator ones column
            nc.gpsimd.memset(
                vaug[:].rearrange("p f (h e) -> p f h e", e=HD + 1)
                [:, :, :, HD:HD + 1], 1.0)
            wt = wqkv["wv"]
            for pt in range(4):
                for cg in range(2):
                    ps = pps.tile([128, 512], F32, tag="ps",
                                  name=f"ps_wv_{pt}_{cg}")
                    for c in range(DC):
                        nc.tensor.matmul(
                            ps[:], xT[:, c, 128 * pt:128 * (pt + 1)],
                            wt[:, c, 512 * cg:512 * (cg + 1)],
                            start=(c == 0), stop=(c == DC - 1))
                    for hh in range(8):
                        h = 8 * cg + hh
                        nc.scalar.copy(
                            vaug[:, pt, (HD + 1) * h:(HD + 1) * h + HD],
                            ps[:, HD * hh:HD * (hh + 1)])
            nc.sync.dma_start(v_loc[:].rearrange("(f p) w -> p f w", p=128),
                              vaug[:])
            nc.gpsimd.collective_compute(
                "AllGather", ALU.bypass, replica_groups=GROUPS,
                ins=[v_loc.opt()], outs=[v_g.opt()])

        # ================= P3: qT (scaled by 1/8) ============================
        proj_headT(qT, wqkv["wq"], bq8_sb, 0.125, "wq")
        es_x.close()  # xT no longer needed
        es_w.close()  # wk/wv/wq no longer needed

        # ================= P4: attention =====================================
        es_attn = ctx.enter_context(contextlib.ExitStack())   # attn_oT: P4..P5
        ao_pool = es_attn.enter_context(tc.tile_pool(name="ao_p", bufs=1, side="left"))
        attn_oT = ao_pool.tile([128, DC, R], BF16, name="attn_oT")
        wp_pool = es_attn.enter_context(tc.tile_pool(name="wp_p", bufs=1, side="left"))
        wp_sb = wp_pool.tile([128, DC, D], BF16, name="w_wp")
        nc.sync.dma_start(
            wp_sb[:], io["wp"][:].rearrange("(c p) d -> p c d", p=128))
        hs5_pool = es_attn.enter_context(
            tc.tile_pool(name="hs5_p", bufs=1, side="left"))
        hst_pre = []
        for rt in range(2):
            t5 = hs5_pool.tile([128, D], F32, name=f"hst5_{rt}")
            nc.sync.dma_start(t5[:], hs[128 * rt:128 * (rt + 1), :])
            hst_pre.append(t5)
        with tc.tile_pool(name="kg_pool", bufs=1, side="left") as kgp, \
             tc.tile_pool(name="vg_pool", bufs=1, side="left") as vgp, \
             tc.tile_pool(name="at_sb", bufs=4, side="left") as asb, \
             tc.tile_pool(name="at_norm", bufs=2, side="left") as anorm, \
             tc.tile_pool(name="oTB_p", bufs=1, side="left") as obp, \
             tc.tile_pool(name="sc_ps", bufs=2, space="PSUM") as scps, \
             tc.tile_pool(name="oT_ps", bufs=4, space="PSUM") as otps:

            hps = (slice(0, 64), slice(64, 128))

            # ---- P4a: diagonal (own-kv) attention from SBUF-resident
            # kT/vaug, runs during the AllGathers; block b lands in q columns
            # [QB*b, QB*(b+1)) of the combined 512-wide partial ----
            oTB_sb = {}
            for hh in range(H // 2):
                h0, h1 = 2 * hh, 2 * hh + 1
                vss = (slice((HD + 1) * h0, (HD + 1) * (h0 + 1)),
                       slice((HD + 1) * h1, (HD + 1) * (h1 + 1)))
                obs = [obp.tile([HD + 1, R], BF16, tag=f"oTB{hh}_{j}",
                                name=f"oTBs_{hh}_{j}") for j in range(2)]
                for b in range(NBLK):
                    qs = slice(QB * b, QB * (b + 1))
                    oTBs = [otps.tile([HD + 1, QB], F32, tag="oT",
                                      name=f"oTB_{b}_{hh}_{j}")
                            for j in range(2)]
                    for i in range(2):
                        sl = 2 * b + i
                        sc = scps.tile([128, 2, 512], F32, tag="sc",
                                       name=f"scB_{b}_{hh}_{i}")
                        scv = sc[:, :, 0:QB]
                        for j in range(2):
                            nc.tensor.matmul(
                                sc[:, j, 0:QB],
                                kT[hps[j], hh, KTW * sl:KTW * (sl + 1)],
                                qT[hps[j], hh, qs],
                                start=True, stop=True)
                        m_sb = maskA_sb if i == 0 else maskB_sb
                        nc.vector.tensor_add(
                            scv, scv,
                            m_sb.rearrange("p (a b) -> p a b", a=2))
                        ex = asb.tile([128, 2, QB], BF16, tag="ex",
                                      name=f"exB_{b}_{hh}_{i}")
                        nc.scalar.activation(ex[:], scv, func=AF.Exp,
                                             bias=biasB_sb[:, b, i:i + 1],
                                             scale=1.0)
                        for j in range(2):
                            nc.tensor.matmul(oTBs[j][:],
                                             vaug[:, sl, vss[j]],
                                             ex[:, j, :],
                                             start=(i == 0), stop=(i == 1))
                    for j in range(2):
                        nc.scalar.copy(obs[j][:, qs], oTBs[j][:])
                for j in range(2):
                    oTB_sb[(hh, j)] = obs[j]

            # kT/vaug end with the diagonal pass; the ex ring reuses their
            # SBUF space (WAR deps keep the reuse safe)
            es_kv.close()
            es_ex = contextlib.ExitStack()
            expool = es_ex.enter_context(
                tc.tile_pool(name="ex_p", bufs=1, side="right"))

            # ---- gathered k/v loads (blocked on the AllGathers); rank 0's
            # upper half is position 7, never attended -> not loaded ----
            kranks, vranks = [], []
            for r in range(4):
                kw = QB if r == 0 else R
                kr = kgp.tile([128, DC, kw], BF16, tag=f"kr{r}",
                              name=f"kr_{r}")
                nc.sync.dma_start(
                    kr[:], k_g[r, :, 0:kw].rearrange("(c p) q -> p c q",
                                                     p=128))
                kranks.append(kr)
                vf = 2 if r == 0 else 4
                vr = vgp.tile([128, vf, VW], BF16, tag=f"vr{r}",
                              name=f"vr_{r}")
                nc.sync.dma_start(
                    vr[:], v_g[r, 0:128 * vf, :].rearrange("(f p) w -> p f w",
                                                           p=128))
                vranks.append(vr)

            def ktile_ap(t, hp, hc):
                r, off = _gtile_src(t)
                return kranks[r][hp, hc, off:off + KTW]

            def vtile_ap(t, vs):
                r, off = _gtile_src(t)
                return vranks[r][:, off // 128, vs]

            # ---- P4b: software-pipelined 14-slot kv sweep, interleaved at
            # slot granularity: qk+exp "score" slots (need only the k gather)
            # run LAG head-pairs ahead of the attn@v slots (which wait on the
            # v gather). Slots 0..5 hit both q-blocks in one 512-wide matmul
            # (block0's depth 2p never exceeds 6); slots 6..13 can only ever
            # feed block1 (depth 14-2p >= 8), so they run 256-wide on its q
            # columns alone. Per-core biasA kills the causally out-of-range
            # remainder.
            LAG = 3
            NWIDE = 6  # slots needing both q-blocks

            def score_slot(hh, t):
                sc = scps.tile([128, 2, R], F32, tag="sc",
                               name=f"scA_{hh}_{t}")
                wq_cols = R if t < NWIDE else QB
                qs = slice(0, R) if t < NWIDE else slice(QB, R)
                for j in range(2):
                    nc.tensor.matmul(sc[:, j, 0:wq_cols],
                                     ktile_ap(t, hps[j], hh),
                                     qT[hps[j], hh, qs],
                                     start=True, stop=True)
                ex = expool.tile([128, 2, wq_cols], BF16,
                                 tag=f"ex{t}_{hh % LAG}",
                                 name=f"exA_{hh}_{t}")
                if t < NWIDE:
                    for half in range(2):
                        qh = slice(QB * half, QB * (half + 1))
                        nc.scalar.activation(
                            ex[:, :, qh], sc[:, :, qh], func=AF.Exp,
                            bias=biasA_sb[:, t, half:half + 1], scale=1.0)
                else:
                    nc.scalar.activation(ex[:], sc[:, :, 0:QB], func=AF.Exp,
                                         bias=biasA_sb[:, t, 1:2], scale=1.0)
                return ex

            def av_slot(hh, t, ex, oTs):
                vss = (slice((HD + 1) * 2 * hh, (HD + 1) * (2 * hh + 1)),
                       slice((HD + 1) * (2 * hh + 1), (HD + 1) * (2 * hh + 2)))
                os_ = slice(0, R) if t < NWIDE else slice(QB, R)
                for j in range(2):
                    nc.tensor.matmul(oTs[j][:, os_], vtile_ap(t, vss[j]),
                                     ex[:, j, :],
                                     start=(t == 0), stop=False,
                                     skip_group_check=True)

            def combine(hh, oTs):
                for j, h in enumerate((2 * hh, 2 * hh + 1)):
                    oT = oTs[j]
                    # fold the diagonal partial in on the PE (identity
                    # matmul closes the psum accumulation group), keeping
                    # the vector engine off the critical path
                    nc.tensor.matmul(oT[:], ident_sb[0:HD + 1, 0:HD + 1],
                                     oTB_sb[(hh, j)][:],
                                     start=False, stop=True,
                                     skip_group_check=True)
                    rec = anorm.tile([1, R], F32, tag="rec", name=f"rec_{h}")
                    nc.vector.reciprocal(rec[:], oT[HD:HD + 1, :])
                    rb = anorm.tile([64, R], F32, tag="rb", name=f"rb_{h}")
                    nc.gpsimd.partition_broadcast(rb[:], rec[:])
                    if j == 0:
                        dst = attn_oT[0:HD, hh, :]
                        nc.vector.tensor_mul(dst, oT[0:HD, :], rb[:])
                        if apply_bv:
                            nc.vector.tensor_scalar_add(
                                dst, dst, bvh_sb[:, h:h + 1])
                    else:
                        tmpn = anorm.tile([64, R], BF16, tag="tmpn",
                                          name=f"tmpn_{h}")
                        nc.vector.tensor_mul(tmpn[:], oT[0:HD, :], rb[:])
                        if apply_bv:
                            nc.vector.tensor_scalar_add(
                                tmpn[:], tmpn[:], bvh_sb[:, h:h + 1])
                        nc.sync.dma_start(attn_oT[64:128, hh, :], tmpn[:])

            pend, live_oTs = {}, {}
            for step in range(H // 2 + LAG):
                if step < H // 2:
                    pend[step] = []
                if step >= LAG:
                    live_oTs[step - LAG] = [
                        otps.tile([HD + 1, R], F32, tag="oT",
                                  name=f"oT_{step - LAG}_{j}")
                        for j in range(2)]
                for t in range(NSLOT):
                    # av before score: the ex slot score(step, t) reuses
                    # (ring of LAG) must have its reader av emitted first
                    if step >= LAG:
                        av_slot(step - LAG, t, pend[step - LAG][t],
                                live_oTs[step - LAG])
                    if step < H // 2:
                        pend[step].append(score_slot(step, t))
                if step >= LAG:
                    combine(step - LAG, live_oTs.pop(step - LAG))
                    del pend[step - LAG]
            es_ex.close()

        # ================= P5+P6: out-proj + residual, fused with LN2 +
        # transpose per row-tile so the LN/transpose latency hides under the
        # next row-tile's projection chains =================================
        es_h = ctx.enter_context(contextlib.ExitStack())      # h_sb: P5..P8
        h_pool = es_h.enter_context(tc.tile_pool(name="h_p", bufs=1, side="right"))
        h_sb = h_pool.tile([128, 4, D], F32)
        es_mlp = ctx.enter_context(contextlib.ExitStack())    # h2T, gT, w2
        mlp_pool = es_mlp.enter_context(tc.tile_pool(name="mlp_p", bufs=1, side="right"))
        h2T = mlp_pool.tile([128, DC, R], BF16)
        gT = mlp_pool.tile([128, GC, R], BF16)
        w2_sb = mlp_pool.tile([128, GC, D], BF16)
        with tc.tile_pool(name="hs2", bufs=2, side="left") as hs2, \
             tc.tile_pool(name="p6", bufs=2, side="left") as p6, \
             tc.tile_pool(name="p6ps", bufs=4, space="PSUM") as p6ps, \
             tc.tile_pool(name="ps_wp", bufs=2, space="PSUM") as pps:
            wt = wp_sb
            for rt in range(4):
                if rt < 2:
                    hst = hst_pre[rt]
                else:
                    hst = hs2.tile([128, D], F32, tag="hst",
                                   name=f"hst2_{rt}")
                    nc.sync.dma_start(hst[:], hs[128 * rt:128 * (rt + 1), :])
                for cg in range(2):
                    ps = pps.tile([128, 512], F32, tag="ps",
                                  name=f"ps_wp_{rt}_{cg}")
                    nc.tensor.matmul(ps[:], ones_r[:],
                                     bpr_sb[:, 512 * cg:512 * (cg + 1)],
                                     start=True, stop=False)
                    for c in range(DC):
                        nc.tensor.matmul(
                            ps[:], attn_oT[:, c, 128 * rt:128 * (rt + 1)],
                            wt[:, c, 512 * cg:512 * (cg + 1)],
                            start=False, stop=(c == DC - 1))
                    nc.vector.tensor_add(
                        h_sb[:, rt, 512 * cg:512 * (cg + 1)],
                        ps[:], hst[:, 512 * cg:512 * (cg + 1)])
                h2 = p6.tile([128, D], BF16, tag="h2")
                layernorm(h2[:], h_sb[:, rt, :], p6, ln2_g, ln2_b)
                transpose_into(h2T, h2, rt, p6ps)
        es_attn.close()  # attn_oT + wp done

        # ================= P7: MLP up + gelu (w2 prefetch underneath) ========
        nc.sync.dma_start(
            w2_sb[:], io["w2"][:].rearrange("(g p) d -> p g d", p=128))
        with tc.tile_pool(name="w_w1", bufs=3, side="left") as wpl, \
             tc.tile_pool(name="ps_w1", bufs=2, space="PSUM") as pps:
            for gc in range(GC):
                wt = wpl.tile([128, DC, 128], BF16, tag="w1")
                nc.sync.dma_start(wt[:], io["w1"][gc])
                ps = pps.tile([128, R], F32, tag="ps", name=f"ps_w1_{gc}")
                for c in range(DC):
                    nc.tensor.matmul(ps[:], wt[:, c, :], h2T[:, c, :],
                                     start=(c == 0), stop=(c == DC - 1))
                nc.scalar.activation(gT[:, gc, :], ps[:], func=AF.Gelu,
                                     bias=b1l_sb[:, gc:gc + 1], scale=1.0)

        # ================= P8: MLP down, qt-major + bias + residual ==========
        with tc.tile_pool(name="o_sb", bufs=2, side="left") as osb, \
             tc.tile_pool(name="o_ps", bufs=2, space="PSUM") as pps:
            for qt in range(4):
                ps = pps.tile([128, 2, 512], F32, tag="ops", name=f"o_ps_{qt}")
                for cg in range(2):
                    nc.tensor.matmul(ps[:, cg, :], ones_r[:],
                                     b2r_sb[:, 512 * cg:512 * (cg + 1)],
                                     start=True, stop=False)
                for gc in range(GC):
                    for cg in range(2):
                        nc.tensor.matmul(
                            ps[:, cg, :],
                            gT[:, gc, 128 * qt:128 * (qt + 1)],
                            w2_sb[:, gc, 512 * cg:512 * (cg + 1)],
                            start=False, stop=(gc == GC - 1))
                ot = osb.tile([128, D], F32, tag="ot", name=f"ot_{qt}")
                nc.vector.tensor_add(ot[:],
                                     ps[:].rearrange("p a b -> p (a b)"),
                                     h_sb[:, qt, :])
                nc.sync.dma_start(out[128 * qt:128 * (qt + 1), :], ot[:])


# ---------------------------------------------------------------------------
# Host side
# ---------------------------------------------------------------------------

_CACHE = {}
LAST_RESULT = None  # BassKernelResults of the most recent run (for test.py)


def _get_program(key):
    if key not in _CACHE:
        _CACHE[key] = build_program(*key)
    return _CACHE[key]


def _colzero_bias(kpos):
    return np.where((kpos % JD) == (JD - 1), np.float32(NEG), np.float32(0.0))


def kernel(hidden_states, Wq, bq, Wk, bk, Wv, bv, Wp, bp,
           ln1_g, ln1_b, ln2_g, ln2_b, W1, b1, W2, b2):
    f32 = lambda a: np.ascontiguousarray(np.asarray(a, dtype=np.float32))
    hidden_states = f32(hidden_states)
    Wq, bq, Wk, bk, Wv, bv, Wp, bp = map(f32, (Wq, bq, Wk, bk, Wv, bv, Wp, bp))
    ln1_g, ln1_b, ln2_g, ln2_b = map(f32, (ln1_g, ln1_b, ln2_g, ln2_b))
    W1, b1, W2, b2 = map(f32, (W1, b1, W2, b2))

    apply_bv = bool(np.any(bv != 0.0))
    apply_ln1 = bool(np.any(ln1_g != 1.0) or np.any(ln1_b != 0.0))
    apply_ln2 = bool(np.any(ln2_g != 1.0) or np.any(ln2_b != 0.0))
    nc = _get_program((apply_bv, apply_ln1, apply_ln2))

    chunk_major = lambda v: np.ascontiguousarray(v.reshape(-1, 128).T)
    kp = np.arange(KTW)[:, None]
    iq = np.arange(KTW)[None, :]
    tri = np.where(kp <= iq, np.float32(0.0), np.float32(NEG))
    maskAB = np.zeros((2, 128, 2 * KTW), dtype=np.float32)
    maskAB[0, :, :KTW] = tri
    maskAB[1, :, :KTW] = NEG
    maskAB[1, :, KTW:] = tri

    import ml_dtypes
    bf = lambda a: np.ascontiguousarray(a.astype(ml_dtypes.bfloat16))
    w1x = np.ascontiguousarray(
        W1.reshape(DC, 128, GC, 128).transpose(2, 1, 0, 3))
    shared = dict(wq=bf(Wq), wk=bf(Wk), wv=bf(Wv), wp=bf(Wp), w1=bf(w1x),
                  w2=bf(W2),
                  bq8=chunk_major(bq * 0.125), bkl=chunk_major(bk),
                  bvh=np.ascontiguousarray(bv.reshape(H, HD).T),
                  b1l=chunk_major(b1), bpr=bf(bp.reshape(1, D)),
                  b2r=bf(b2.reshape(1, D)), ln1gb=np.stack([ln1_g, ln1_b]),
                  ln2gb=np.stack([ln2_g, ln2_b]), maskAB=maskAB,
                  ident=np.eye(128, dtype=ml_dtypes.bfloat16),
                  onesr=np.ones((1, 128), dtype=ml_dtypes.bfloat16))

    in_maps, row_map = [], []
    for core in range(NCORE):
        # cores 0-3: batch 0, positions (p, 7-p); cores 4-7: batch 1 mirror
        batch, p = core // 4, core % 4
        positions = (p, 7 - p)
        rows = [np.arange(QB * pb, QB * (pb + 1)) for pb in positions]
        row_map.append((batch, rows))
        depths = (2 * p, 14 - 2 * p)   # pass-A kv tiles needed per block

        biasA = np.empty((128, NSLOT, 2), dtype=np.float32)
        for t in range(NSLOT):
            cz = _colzero_bias(KTW * t + np.arange(KTW))
            for half in range(2):
                biasA[:, t, half] = cz if t < depths[half] else NEG
        biasB = np.zeros((128, NBLK, 2), dtype=np.float32)
        for b, pb in enumerate(positions):
            for i in range(2):
                biasB[:, b, i] = _colzero_bias(QB * pb + KTW * i
                                               + np.arange(KTW))

        m = dict(shared)
        m["hs"] = np.ascontiguousarray(
            np.concatenate([hidden_states[batch, rows[0], :],
                            hidden_states[batch, rows[1], :]]))
        m["biasA"] = np.ascontiguousarray(biasA)
        m["biasB"] = np.ascontiguousarray(biasB)
        in_maps.append(m)

    res = run_bass_kernel_spmd(nc, in_maps, core_ids=list(range(NCORE)))
    global LAST_RESULT
    LAST_RESULT = res

    out_full = np.empty((B, S, D), dtype=np.float32)
    for core in range(NCORE):
        batch, rows = row_map[core]
        o = res.results[core]["out"]
        out_full[batch, rows[0], :] = o[:QB]
        out_full[batch, rows[1], :] = o[QB:]
    return out_full
